# revision 22
# baseline (speedup 1.0000x reference)
"""Trainium2 Bass kernel for a ViT-style transformer block (B=4, N=1370, C=1024).

Sharding: 8 cores = 4 batches x 2 token-halves. Each core runs the full block
for its 685 query tokens; K/V are computed for all 1370 tokens of its batch
(no collectives needed). The token-half selection is done by rolling the token
axis on the host so every core runs an identical program on tokens [0, 685).

On-chip layout: activations are kept feature-on-partition ("transposed",
[C, tokens]) the whole way through:
  - layernorm stats (sum, sum of squares over C) via ones-matmul on the PE,
    with lhsT = ones[128,128] so the stats are partition-broadcast for free
  - per-channel affines (ln gamma/beta, biases, layer-scale gammas) are
    per-partition scalars (native tensor_scalar broadcast)
  - attention computes S^T = K @ Q^T per head; exp on ScalarE directly from
    PSUM; A@V is lhsT=[V|ones] so the softmax denominator rides along as one
    extra output row; normalization via reciprocal + partition-broadcast DMA
Weights are host-pretransposed/pretiled so every DMA is contiguous, and the
layer-scale gammas (1e-5) plus the attention 1/sqrt(dh) are folded into the
weights/biases on the host.
"""

import numpy as np
import ml_dtypes

import concourse.bass as bass
import concourse.mybir as mybir
import concourse.tile as tile
from concourse.bass_utils import run_bass_kernel_spmd

B, N, C = 4, 1370, 1024
H, DH, HID = 16, 64, 4096
P = 128
CT = C // P            # 8 feature tiles
HT = HID // P          # 32 hidden tiles
NCORES = 8
Q = N // 2             # 685 query tokens per core
KT = (N + P - 1) // P  # 11 key-token tiles (last has 90 rows)
EPS = 1e-5

F32 = mybir.dt.float32
BF16 = mybir.dt.bfloat16
ADD = mybir.AluOpType.add
SUB = mybir.AluOpType.subtract
MUL = mybir.AluOpType.mult
AF = mybir.ActivationFunctionType


def _chunks(total, size):
    out = []
    off = 0
    while off < total:
        out.append((off, min(size, total - off)))
        off += size
    return out


QCH = _chunks(Q, 512)   # query-token chunks
TCH = _chunks(N, 512)   # full-token chunks


def _pbroadcast(ap, n):
    """Partition-broadcast an AP whose partition dim is 1 to n partitions."""
    return bass.AP(tensor=ap.tensor, offset=ap.offset, ap=[[0, n]] + list(ap.ap[1:]))


def _layernorm(nc, work, psum, src_of, ntok, g_sb, b_sb, eps_sb, ones, out_ht,
               csz=512):
    """LN over the feature axis (partitions). src_of(off, n) -> fp32 AP [P, CT, n].
    Writes normalized bf16 into out_ht[:, k, off:off+n]."""
    for (toff, tn) in _chunks(ntok, csz):
        xc = src_of(toff, tn)
        ps_sx = psum.tile([P, 512], F32, tag="ps")
        ps_sx2 = psum.tile([P, 512], F32, tag="ps")
        for k in range(CT):
            x2 = work.tile([P, csz], F32, tag="ln_x2")
            nc.vector.tensor_mul(x2[:, :tn], xc[:, k, :], xc[:, k, :])
            nc.tensor.matmul(ps_sx[:, :tn], ones, xc[:, k, :],
                             start=(k == 0), stop=(k == CT - 1))
            nc.tensor.matmul(ps_sx2[:, :tn], ones, x2[:, :tn],
                             start=(k == 0), stop=(k == CT - 1))
        mean = work.tile([P, csz], F32, tag="ln_mean")
        nc.vector.tensor_scalar_mul(mean[:, :tn], ps_sx[:, :tn], 1.0 / C)
        m2 = work.tile([P, csz], F32, tag="ln_m2")
        nc.vector.tensor_mul(m2[:, :tn], mean[:, :tn], mean[:, :tn])
        rstd = work.tile([P, csz], F32, tag="ln_rstd")
        nc.vector.scalar_tensor_tensor(rstd[:, :tn], ps_sx2[:, :tn], 1.0 / C,
                                       m2[:, :tn], MUL, SUB)
        nc.scalar.activation(rstd[:, :tn], rstd[:, :tn], AF.Sqrt,
                             bias=eps_sb, scale=1.0)
        nc.vector.reciprocal(rstd[:, :tn], rstd[:, :tn])
        for k in range(CT):
            xm = work.tile([P, csz], F32, tag="ln_xm")
            nc.vector.tensor_tensor(xm[:, :tn], xc[:, k, :], mean[:, :tn], SUB)
            tg = work.tile([P, csz], F32, tag="ln_tg")
            nc.vector.scalar_tensor_tensor(tg[:, :tn], xm[:, :tn],
                                           g_sb[:, k:k + 1], rstd[:, :tn],
                                           MUL, MUL)
            nc.vector.tensor_scalar_add(out_ht[:, k, toff:toff + tn],
                                        tg[:, :tn], b_sb[:, k:k + 1])


_WAIT_EXEMPT = {
    "InstEventSemaphore", "InstNoOp",
    "InstCall", "InstBranchHint", "InstHalt", "InstCollectiveCompute",
}


def _legalize_matmul_waits(nc):
    """This walrus build allows only ONE sync wait per compute instruction.
    Move extra waits onto NoOps inserted immediately before the instruction
    (same engine stream position => identical ordering semantics)."""
    nid = [0]
    for fn in nc.m.functions:
        for blk in fn.blocks:
            insts = blk.instructions
            i = 0
            while i < len(insts):
                ins = insts[i]
                tname = type(ins).__name__
                si = getattr(ins, "sync_info", None)
                if (tname not in _WAIT_EXEMPT and tname.startswith("Inst")
                        and si is not None and len(si.on_wait) > 1):
                    waits = list(si.on_wait)
                    for w in waits[:-1]:
                        nop = mybir.InstNoOp(
                            name=f"I-mmwait-{nid[0]}", engine=ins.engine,
                            ins=[], outs=[],
                            sync_info=mybir.SyncInfo(on_wait=[w],
                                                     on_update=[]))
                        nid[0] += 1
                        insts.insert(i, nop)
                        i += 1
                    ins.sync_info = mybir.SyncInfo(on_wait=[waits[-1]],
                                                   on_update=si.on_update)
                i += 1


def _build_program():
    nc = bass.Bass()
    d = {}
    d["xt"] = nc.declare_dram_parameter("xt", [P, CT, N], F32, isOutput=False)
    d["wqk"] = nc.declare_dram_parameter("wqk", [16, P, CT, P], BF16, isOutput=False)
    d["bqk"] = nc.declare_dram_parameter("bqk", [P, 16], F32, isOutput=False)
    d["wv"] = nc.declare_dram_parameter("wv", [P, CT, C], BF16, isOutput=False)
    d["wproj"] = nc.declare_dram_parameter("wproj", [CT, P, CT, P], BF16, isOutput=False)
    d["bproj"] = nc.declare_dram_parameter("bproj", [P, CT], F32, isOutput=False)
    d["ln1g"] = nc.declare_dram_parameter("ln1g", [P, CT], F32, isOutput=False)
    d["ln1b"] = nc.declare_dram_parameter("ln1b", [P, CT], F32, isOutput=False)
    d["ln2g"] = nc.declare_dram_parameter("ln2g", [P, CT], F32, isOutput=False)
    d["ln2b"] = nc.declare_dram_parameter("ln2b", [P, CT], F32, isOutput=False)
    d["wfc1"] = nc.declare_dram_parameter("wfc1", [HT, P, CT, P], BF16, isOutput=False)
    d["bfc1"] = nc.declare_dram_parameter("bfc1", [P, HT], F32, isOutput=False)
    d["wfc2"] = nc.declare_dram_parameter("wfc2", [CT, P, HT, P], BF16, isOutput=False)
    d["bfc2"] = nc.declare_dram_parameter("bfc2", [P, CT], F32, isOutput=False)
    out_d = nc.declare_dram_parameter("out", [P, CT, Q], F32, isOutput=True)

    with tile.TileContext(nc) as tc:
        with tc.tile_pool(name="const", bufs=1) as const, \
             tc.tile_pool(name="persist", bufs=1) as persist:
            ones = const.tile([P, P], F32)
            nc.vector.memset(ones, 1.0)
            eps_sb = const.tile([P, 1], F32)
            nc.vector.memset(eps_sb, EPS)

            def load_const(name, shape):
                t = const.tile(shape, F32, tag=f"const_{name}")
                nc.sync.dma_start(t, d[name][:, :])
                return t

            ln1g_sb = load_const("ln1g", [P, CT])
            ln1b_sb = load_const("ln1b", [P, CT])
            ln2g_sb = load_const("ln2g", [P, CT])
            ln2b_sb = load_const("ln2b", [P, CT])
            bqk_sb = load_const("bqk", [P, 16])
            bproj_sb = load_const("bproj", [P, CT])
            bfc1_sb = load_const("bfc1", [P, HT])
            bfc2_sb = load_const("bfc2", [P, CT])

            hT = persist.tile([P, CT, N], BF16)      # ln1 output, all tokens
            QTt = persist.tile([P, CT, Q], BF16)     # Q^T (scaled by dh^-0.5)
            KTt = persist.tile([P, CT, N], BF16)     # K^T
            vaug = persist.tile([P, KT, H, DH + 1], BF16)  # V | ones, token-partition
            oT = persist.tile([P, CT, Q], BF16)      # attention out, normalized
            x1T = persist.tile([P, CT, Q], F32)      # residual after attention
            h2T = persist.tile([P, CT, Q], BF16)     # ln2 output

            nc.vector.memset(vaug[:, :, :, DH:DH + 1], 1.0)

            # warmup matmul so the PE clock observes the DVE memsets before
            # any data matmul (walrus allows only one sync wait per Matmult)
            with tc.tile_pool(name="warm", bufs=1, space="PSUM") as warm:
                wps = warm.tile([P, P], F32)
                nc.tensor.matmul(wps, ones, ones, start=True, stop=True)

            # ---------- Phase A: LN1 over all 1370 tokens ----------
            with tc.tile_pool(name="lnw", bufs=2) as lnw, \
                 tc.tile_pool(name="psln", bufs=4, space="PSUM") as psln:
                def src_ln1(toff, tn):
                    xc = lnw.tile([P, CT, 256], F32, tag="ln_xc")
                    nc.sync.dma_start(xc[:, :, :tn], d["xt"][:, :, toff:toff + tn])
                    return xc[:, :, :tn]
                _layernorm(nc, lnw, psln, src_ln1, N, ln1g_sb, ln1b_sb,
                           eps_sb, ones, hT, csz=256)

            # ---------- Phase B: QKV projections ----------
            with tc.tile_pool(name="wqp", bufs=2) as wqp, \
                 tc.tile_pool(name="wvp", bufs=1) as wvp, \
                 tc.tile_pool(name="psqkv", bufs=4, space="PSUM") as psqkv:
                for m in range(16):
                    wm = wqp.tile([P, CT, P], BF16, tag="wm")
                    nc.sync.dma_start(wm, d["wqk"][m])
                    dest = QTt if m < 8 else KTt
                    chs = QCH if m < 8 else TCH
                    for (qoff, qn) in chs:
                        ps = psqkv.tile([P, 512], F32, tag="ps")
                        for k in range(CT):
                            nc.tensor.matmul(ps[:, :qn], wm[:, k, :],
                                             hT[:, k, qoff:qoff + qn],
                                             start=(k == 0), stop=(k == CT - 1))
                        nc.vector.tensor_scalar_add(dest[:, m % 8, qoff:qoff + qn],
                                                    ps[:, :qn], bqk_sb[:, m:m + 1])
                wv_sb = wvp.tile([P, CT, C], BF16)
                nc.sync.dma_start(wv_sb, d["wv"][:, :, :])
                for t in range(KT):
                    tp = min(P, N - t * P)
                    for vc in range(2):
                        ps = psqkv.tile([P, 512], F32, tag="ps")
                        for k in range(CT):
                            nc.tensor.matmul(ps[:tp, :], hT[:, k, t * P:t * P + tp],
                                             wv_sb[:, k, vc * 512:(vc + 1) * 512],
                                             start=(k == 0), stop=(k == CT - 1))
                        # evac on ACT so AV matmuls depend on one engine only
                        # (b_v is folded into the proj bias on the host)
                        nc.scalar.copy(
                            vaug[:tp, t, vc * 8:(vc + 1) * 8, :DH],
                            ps[:tp, :].rearrange("p (h dh) -> p h dh", dh=DH))

            # ---------- Phase C: attention ----------
            GROUPS = [[0, 1, 2], [3, 4, 5], [6, 7, 8], [9, 10]]
            with tc.tile_pool(name="pss", bufs=2, space="PSUM") as pss, \
                 tc.tile_pool(name="psav", bufs=2, space="PSUM") as psav, \
                 tc.tile_pool(name="ptp", bufs=3) as ptp, \
                 tc.tile_pool(name="nrmd", bufs=2, space="DRAM") as nrmd, \
                 tc.tile_pool(name="nrm", bufs=2) as nrm:
                for h in range(H):
                    ht_, hoff = h // 2, (h % 2) * DH
                    for (qoff, qn) in QCH:
                        ps_av = psav.tile([P, 512], F32, tag="av")
                        pending = []  # (pt_tile, group)

                        def flush_av(pending):
                            pt, grp = pending.pop(0)
                            for jj, j in enumerate(grp):
                                tp = min(P, N - j * P)
                                nc.tensor.matmul(ps_av[:DH + 1, :qn],
                                                 vaug[:tp, j, h, :],
                                                 pt[:tp, jj, :qn],
                                                 start=(j == 0), stop=(j == KT - 1))

                        for gi, grp in enumerate(GROUPS):
                            ps_s = pss.tile([P, 3, 512], F32, tag="s")
                            for jj, j in enumerate(grp):
                                tp = min(P, N - j * P)
                                nc.tensor.matmul(
                                    ps_s[:tp, jj, :qn],
                                    KTt[hoff:hoff + DH, ht_, j * P:j * P + tp],
                                    QTt[hoff:hoff + DH, ht_, qoff:qoff + qn],
                                    start=True, stop=True)
                            pt = ptp.tile([P, 3, 512], BF16, tag="pt")
                            if gi < len(GROUPS) - 1:
                                nc.scalar.activation(pt[:, :len(grp), :qn],
                                                     ps_s[:, :len(grp), :qn], AF.Exp)
                            else:
                                # ragged last group: avoid reading unwritten psum rows
                                nc.scalar.activation(pt[:, 0:1, :qn],
                                                     ps_s[:, 0:1, :qn], AF.Exp)
                                tp = N - 10 * P
                                nc.scalar.activation(pt[:tp, 1:2, :qn],
                                                     ps_s[:tp, 1:2, :qn], AF.Exp)
                            pending.append((pt, grp))
                            if len(pending) > 1:
                                flush_av(pending)
                        while pending:
                            flush_av(pending)
                        # normalize: out rows = ps_av[:DH] / ps_av[DH]
                        rrow = nrm.tile([1, 512], F32, tag="rrow")
                        nc.vector.reciprocal(rrow[:, :qn], ps_av[DH:DH + 1, :qn])
                        rdram = nrmd.tile([1, 512], F32, tag="rd")
                        nc.sync.dma_start(rdram[:, :qn], rrow[:, :qn])
                        rbc = nrm.tile([DH, 512], F32, tag="rbc")
                        nc.sync.dma_start(rbc[:, :qn],
                                          _pbroadcast(rdram[:1, :qn], DH))
                        nc.vector.tensor_tensor(oT[hoff:hoff + DH, ht_, qoff:qoff + qn],
                                                ps_av[:DH, :qn], rbc[:, :qn], MUL)

            # ---------- Phase D: proj + residual + LN2 ----------
            with tc.tile_pool(name="prw", bufs=2) as prw, \
                 tc.tile_pool(name="xqp", bufs=1) as xqp, \
                 tc.tile_pool(name="pspr", bufs=4, space="PSUM") as pspr:
                xq = xqp.tile([P, CT, Q], F32)
                nc.sync.dma_start(xq, d["xt"][:, :, :Q])
                for m in range(CT):
                    wm = prw.tile([P, CT, P], BF16, tag="wm")
                    nc.sync.dma_start(wm, d["wproj"][m])
                    for (qoff, qn) in QCH:
                        ps = pspr.tile([P, 512], F32, tag="ps")
                        for k in range(CT):
                            nc.tensor.matmul(ps[:, :qn], wm[:, k, :],
                                             oT[:, k, qoff:qoff + qn],
                                             start=(k == 0), stop=(k == CT - 1))
                        nc.vector.scalar_tensor_tensor(
                            x1T[:, m, qoff:qoff + qn], ps[:, :qn],
                            bproj_sb[:, m:m + 1], xq[:, m, qoff:qoff + qn],
                            ADD, ADD)
                _layernorm(nc, prw, pspr,
                           lambda toff, tn: x1T[:, :, toff:toff + tn],
                           Q, ln2g_sb, ln2b_sb, eps_sb, ones, h2T)

            # ---------- Phase E: MLP + residual ----------
            with tc.tile_pool(name="f1w", bufs=2) as f1w, \
                 tc.tile_pool(name="f2w", bufs=2) as f2w, \
                 tc.tile_pool(name="gel", bufs=1) as gel, \
                 tc.tile_pool(name="outp", bufs=2) as outp, \
                 tc.tile_pool(name="psml", bufs=4, space="PSUM") as psml:
                geluT = gel.tile([P, HT, Q], BF16)
                for m in range(HT):
                    wm = f1w.tile([P, CT, P], BF16, tag="w1")
                    nc.sync.dma_start(wm, d["wfc1"][m])
                    for (qoff, qn) in QCH:
                        ps = psml.tile([P, 512], F32, tag="ps")
                        for k in range(CT):
                            nc.tensor.matmul(ps[:, :qn], wm[:, k, :],
                                             h2T[:, k, qoff:qoff + qn],
                                             start=(k == 0), stop=(k == CT - 1))
                        nc.scalar.activation(geluT[:, m, qoff:qoff + qn],
                                             ps[:, :qn], AF.Gelu,
                                             bias=bfc1_sb[:, m:m + 1], scale=1.0)
                for m in range(CT):
                    wm = f2w.tile([P, HT, P], BF16, tag="w2")
                    nc.sync.dma_start(wm, d["wfc2"][m])
                    om = outp.tile([P, Q], F32, tag="om")
                    for (qoff, qn) in QCH:
                        ps = psml.tile([P, 512], F32, tag="ps")
                        for k in range(HT):
                            nc.tensor.matmul(ps[:, :qn], wm[:, k, :],
                                             geluT[:, k, qoff:qoff + qn],
                                             start=(k == 0), stop=(k == HT - 1))
                        nc.vector.scalar_tensor_tensor(
                            om[:, qoff:qoff + qn], ps[:, :qn],
                            bfc2_sb[:, m:m + 1], x1T[:, m, qoff:qoff + qn],
                            ADD, ADD)
                    nc.sync.dma_start(out_d[:, m, :], om[:, :])

    _legalize_matmul_waits(nc)
    return nc


_PROGRAM = None


def _get_program():
    global _PROGRAM
    if _PROGRAM is None:
        _PROGRAM = _build_program()
    return _PROGRAM


def _ptile(w, n_out_tiles, n_in_tiles):
    """[Cin, Cout] -> [m, p, k, col] pretiled lhsT layout, bf16."""
    a = w.reshape(n_in_tiles, P, n_out_tiles, P)
    return np.ascontiguousarray(a.transpose(2, 1, 0, 3)).astype(ml_dtypes.bfloat16)


def _col_layout(v):
    """[D] -> [P, D//P] with column j = dims j*128..j*128+127."""
    return np.ascontiguousarray(v.reshape(-1, P).T).astype(np.float32)


def prepare_inputs(x, ln1_g, ln1_b, w_qkv, b_qkv, w_proj, b_proj, gamma1,
                   ln2_g, ln2_b, w_fc1, b_fc1, w_fc2, b_fc2, gamma2):
    """Host-side prep: returns (shared weight map, per-core input maps)."""
    scale = DH ** -0.5
    wqkvT = np.ascontiguousarray(w_qkv.T).astype(np.float32)  # [C, 3C]
    wqkvT[:, :C] *= scale
    b_qkv = np.asarray(b_qkv, np.float32).copy()
    b_qkv[:C] *= scale

    wm = {}
    wm["wqk"] = _ptile(wqkvT[:, :2 * C], 16, CT)
    wm["bqk"] = _col_layout(b_qkv[:2 * C])
    wv = np.ascontiguousarray(wqkvT[:, 2 * C:])  # [C, C]
    wm["wv"] = np.ascontiguousarray(
        wv.reshape(CT, P, C).transpose(1, 0, 2)).astype(ml_dtypes.bfloat16)
    wprojT = (np.asarray(w_proj, np.float32).T * np.asarray(gamma1, np.float32)[None, :])
    wm["wproj"] = _ptile(wprojT, CT, CT)
    # b_v passes through softmax unchanged (convex combination), fold it here
    b_v = b_qkv[2 * C:]
    bproj_eff = (np.asarray(b_proj, np.float32)
                 + b_v @ np.asarray(w_proj, np.float32).T)
    wm["bproj"] = _col_layout(bproj_eff * np.asarray(gamma1, np.float32))
    wm["ln1g"] = _col_layout(np.asarray(ln1_g, np.float32))
    wm["ln1b"] = _col_layout(np.asarray(ln1_b, np.float32))
    wm["ln2g"] = _col_layout(np.asarray(ln2_g, np.float32))
    wm["ln2b"] = _col_layout(np.asarray(ln2_b, np.float32))
    wm["wfc1"] = _ptile(np.ascontiguousarray(np.asarray(w_fc1, np.float32).T), HT, CT)
    wm["bfc1"] = _col_layout(np.asarray(b_fc1, np.float32))
    wfc2T = (np.asarray(w_fc2, np.float32).T * np.asarray(gamma2, np.float32)[None, :])
    wm["wfc2"] = _ptile(wfc2T, CT, HT)
    wm["bfc2"] = _col_layout(np.asarray(b_fc2, np.float32) * np.asarray(gamma2, np.float32))

    in_maps = []
    x = np.asarray(x, np.float32)
    for core in range(NCORES):
        b, t = core // 2, core % 2
        xb = np.roll(x[b], -t * Q, axis=0)  # queries become tokens [0, Q)
        xtl = np.ascontiguousarray(
            xb.T.reshape(CT, P, N).transpose(1, 0, 2)).astype(np.float32)
        m = dict(wm)
        m["xt"] = xtl
        in_maps.append(m)
    return in_maps


def gather_output(results):
    out = np.empty((B, N, C), np.float32)
    for core in range(NCORES):
        b, t = core // 2, core % 2
        o = results[core]["out"]  # [P, CT, Q]
        out[b, t * Q:(t + 1) * Q, :] = o.transpose(1, 0, 2).reshape(C, Q).T
    return out


def kernel(**inputs):
    nc = _get_program()
    in_maps = prepare_inputs(**{k: np.asarray(v) for k, v in inputs.items()})
    res = run_bass_kernel_spmd(nc, in_maps, list(range(NCORES)))
    return gather_output(res.results)


if __name__ == "__main__":
    _get_program()
    print("program built OK")


# revision 56
# speedup vs baseline: 292.2740x; 292.2740x over previous
"""Trainium2 Bass kernel for a ViT-style transformer block (B=4, N=1370, C=1024).

Sharding: 8 cores = 4 batches x 2 token-halves. Each core runs the full block
for its 685 query tokens; K/V are computed for all 1370 tokens of its batch
(no collectives needed). The token-half selection is done by rolling the token
axis on the host so every core runs an identical program on tokens [0, 685).

On-chip layout: activations are kept feature-on-partition ("transposed",
[C, tokens]) the whole way through:
  - layernorm stats (sum, sum of squares over C) via ones-matmul on the PE,
    with lhsT = ones[128,128] so the stats are partition-broadcast for free
  - per-channel affines (ln gamma/beta, biases, layer-scale gammas) are
    per-partition scalars (native tensor_scalar broadcast)
  - attention computes S^T = K @ Q^T per head; exp on ScalarE directly from
    PSUM; A@V is lhsT=[V|ones] so the softmax denominator rides along as one
    extra output row; normalization via reciprocal + partition-broadcast DMA
Weights are host-pretransposed/pretiled so every DMA is contiguous, and the
layer-scale gammas (1e-5) plus the attention 1/sqrt(dh) are folded into the
weights/biases on the host.
"""

import numpy as np
import ml_dtypes

import concourse.bass as bass
import concourse.mybir as mybir
import concourse.tile as tile
from concourse.bass_utils import run_bass_kernel_spmd

B, N, C = 4, 1370, 1024
H, DH, HID = 16, 64, 4096
P = 128
CT = C // P            # 8 feature tiles
HT = HID // P          # 32 hidden tiles
NCORES = 8
Q = N // 2             # 685 query tokens per core
KT = (N + P - 1) // P  # 11 key-token tiles (last has 90 rows)
EPS = 1e-5

F32 = mybir.dt.float32
BF16 = mybir.dt.bfloat16
F8 = mybir.dt.float8e4
F8NP = mybir.dt.np(F8)
WS = 256.0           # fp8 weight scale (0.02-scale weights are denormal in e4m3)
NP = 1376            # N padded to a 16 multiple (fp8 DoubleRow stride rule)
QP = 688             # Q padded likewise
ADD = mybir.AluOpType.add
SUB = mybir.AluOpType.subtract
MUL = mybir.AluOpType.mult
AF = mybir.ActivationFunctionType


def _chunks(total, size):
    out = []
    off = 0
    while off < total:
        out.append((off, min(size, total - off)))
        off += size
    return out


QCH = _chunks(Q, 512)   # query-token chunks
TCH = _chunks(N, 512)   # full-token chunks


def _pbroadcast(ap, n):
    """Partition-broadcast an AP whose partition dim is 1 to n partitions."""
    return bass.AP(tensor=ap.tensor, offset=ap.offset, ap=[[0, n]] + list(ap.ap[1:]))


def _layernorm(nc, work, psum, src_of, ntok, g_sb, b_sb, eps_sb, ones, out_ht,
               csz=512, chunk_list=None):
    """LN over the feature axis (partitions). src_of(off, n) -> fp32 AP [P, CT, n].
    Writes normalized output into out_ht[:, k, off:off+n]."""
    chunks = chunk_list if chunk_list is not None else _chunks(ntok, csz)
    for (toff, tn) in chunks:
        xc = src_of(toff, tn)
        ps_sx = psum.tile([P, 512], F32, tag="ps")
        ps_sx2 = psum.tile([P, 512], F32, tag="ps")
        for k in range(CT):
            xb = work.tile([P, csz], BF16, tag="ln_xb")
            nc.gpsimd.tensor_copy(xb[:, :tn], xc[:, k, :])
            x2 = work.tile([P, csz], BF16, tag="ln_x2")
            nc.scalar.activation(x2[:, :tn], xc[:, k, :], AF.Square)
            nc.tensor.matmul(ps_sx[:, :tn], ones, xb[:, :tn],
                             start=(k == 0), stop=(k == CT - 1))
            nc.tensor.matmul(ps_sx2[:, :tn], ones, x2[:, :tn],
                             start=(k == 0), stop=(k == CT - 1))
        mean = work.tile([P, csz], F32, tag="ln_mean")
        nc.vector.tensor_scalar_mul(mean[:, :tn], ps_sx[:, :tn], 1.0 / C)
        rstd = work.tile([P, csz], F32, tag="ln_rstd")
        nc.vector.tensor_mul(rstd[:, :tn], mean[:, :tn], mean[:, :tn])
        nc.vector.scalar_tensor_tensor(rstd[:, :tn], ps_sx2[:, :tn], 1.0 / C,
                                       rstd[:, :tn], MUL, SUB)
        nc.scalar.activation(rstd[:, :tn], rstd[:, :tn], AF.Sqrt,
                             bias=eps_sb, scale=1.0)
        nc.vector.reciprocal(rstd[:, :tn], rstd[:, :tn])
        for k in range(CT):
            xm = work.tile([P, csz], F32, tag="ln_xm")
            nc.vector.tensor_tensor(xm[:, :tn], xc[:, k, :], mean[:, :tn], SUB)
            nc.vector.scalar_tensor_tensor(xm[:, :tn], xm[:, :tn],
                                           g_sb[:, k:k + 1], rstd[:, :tn],
                                           MUL, MUL)
            nc.vector.tensor_scalar_add(out_ht[:, k, toff:toff + tn],
                                        xm[:, :tn], b_sb[:, k:k + 1])


_WAIT_EXEMPT = {
    "InstEventSemaphore", "InstNoOp",
    "InstCall", "InstBranchHint", "InstHalt", "InstCollectiveCompute",
}


def _legalize_matmul_waits(nc):
    """This walrus build allows only ONE sync wait per compute instruction.
    Move extra waits onto NoOps inserted immediately before the instruction
    (same engine stream position => identical ordering semantics)."""
    nid = [0]
    for fn in nc.m.functions:
        for blk in fn.blocks:
            insts = blk.instructions
            i = 0
            while i < len(insts):
                ins = insts[i]
                tname = type(ins).__name__
                si = getattr(ins, "sync_info", None)
                if (tname not in _WAIT_EXEMPT and tname.startswith("Inst")
                        and si is not None and len(si.on_wait) > 1):
                    waits = list(si.on_wait)
                    for w in waits[:-1]:
                        nop = mybir.InstNoOp(
                            name=f"I-mmwait-{nid[0]}", engine=ins.engine,
                            ins=[], outs=[],
                            sync_info=mybir.SyncInfo(on_wait=[w],
                                                     on_update=[]))
                        nid[0] += 1
                        insts.insert(i, nop)
                        i += 1
                    ins.sync_info = mybir.SyncInfo(on_wait=[waits[-1]],
                                                   on_update=si.on_update)
                i += 1


def _build_program():
    nc = bass.Bass()
    d = {}
    d["xt"] = nc.declare_dram_parameter("xt", [P, CT, N], F32, isOutput=False)
    d["wqk"] = nc.declare_dram_parameter("wqk", [16, P, CT, P], F8, isOutput=False)
    d["bqk"] = nc.declare_dram_parameter("bqk", [P, 16], F32, isOutput=False)
    d["wv"] = nc.declare_dram_parameter("wv", [P, CT, C], F8, isOutput=False)
    d["wproj"] = nc.declare_dram_parameter("wproj", [P, CT, C], F8, isOutput=False)
    d["g1s"] = nc.declare_dram_parameter("g1s", [P, CT], F32, isOutput=False)
    d["g2s"] = nc.declare_dram_parameter("g2s", [P, CT], F32, isOutput=False)
    d["bproj"] = nc.declare_dram_parameter("bproj", [P, CT], F32, isOutput=False)
    d["ln1g"] = nc.declare_dram_parameter("ln1g", [P, CT], F32, isOutput=False)
    d["ln1b"] = nc.declare_dram_parameter("ln1b", [P, CT], F32, isOutput=False)
    d["ln2g"] = nc.declare_dram_parameter("ln2g", [P, CT], F32, isOutput=False)
    d["ln2b"] = nc.declare_dram_parameter("ln2b", [P, CT], F32, isOutput=False)
    d["wfc1"] = nc.declare_dram_parameter("wfc1", [P, CT, HID], F8, isOutput=False)
    d["bfc1"] = nc.declare_dram_parameter("bfc1", [P, HT], F32, isOutput=False)
    d["wfc2"] = nc.declare_dram_parameter("wfc2", [CT, P, HT, P], F8, isOutput=False)
    d["bfc2"] = nc.declare_dram_parameter("bfc2", [P, CT], F32, isOutput=False)
    out_d = nc.declare_dram_parameter("out", [P, CT, Q], F32, isOutput=True)

    with tile.TileContext(nc) as tc:
        with tc.tile_pool(name="const", bufs=1) as const, \
             tc.tile_pool(name="persist", bufs=1) as persist:
            ones = const.tile([P, P], BF16)
            nc.vector.memset(ones, 1.0)
            eps_sb = const.tile([P, 1], F32)
            nc.vector.memset(eps_sb, EPS)

            def load_const(name, shape):
                t = const.tile(shape, F32, tag=f"const_{name}")
                nc.sync.dma_start(t, d[name][:, :])
                return t

            ln1g_sb = load_const("ln1g", [P, CT])
            ln1b_sb = load_const("ln1b", [P, CT])
            ln2g_sb = load_const("ln2g", [P, CT])
            ln2b_sb = load_const("ln2b", [P, CT])
            bqk_sb = load_const("bqk", [P, 16])
            bproj_sb = load_const("bproj", [P, CT])
            g1s_sb = load_const("g1s", [P, CT])
            g2s_sb = load_const("g2s", [P, CT])
            bfc1_sb = load_const("bfc1", [P, HT])
            bfc2_sb = load_const("bfc2", [P, CT])

            hT = persist.tile([P, CT, NP], F8)       # ln1 output, all tokens
            QTt = persist.tile([P, CT, Q], BF16)     # Q^T (scaled by dh^-0.5)
            KTt = persist.tile([P, CT, N], BF16)     # K^T
            vaug = persist.tile([P, KT, H, DH + 1], BF16)  # V | ones, token-partition
            oT = persist.tile([P, CT, QP], F8)       # attention out, normalized
            x1T = persist.tile([P, CT, Q], F32)      # residual after attention
            h2T = persist.tile([P, CT, QP], F8)      # ln2 output

            nc.vector.memset(vaug[:, :, :, DH:DH + 1], 1.0)

            # warmup matmul so the PE clock observes the DVE memsets before
            # any data matmul (walrus allows only one sync wait per Matmult)
            with tc.tile_pool(name="warm", bufs=1, space="PSUM") as warm:
                wps = warm.tile([P, P], F32)
                nc.tensor.matmul(wps, ones, ones, start=True, stop=True)

            # ---------- Phase A+B: LN1 + QKV projections ----------
            with tc.tile_pool(name="lnw", bufs=2) as lnw, \
                 tc.tile_pool(name="wqp", bufs=16) as wqp, \
                 tc.tile_pool(name="wvp", bufs=1) as wvp, \
                 tc.tile_pool(name="psln1", bufs=2, space="PSUM") as psln1, \
                 tc.tile_pool(name="psA", bufs=2, space="PSUM") as psA, \
                 tc.tile_pool(name="psV", bufs=2, space="PSUM") as psV:
                # hoist weight DMAs ahead of the x-chunk DMAs so the first
                # QK matmuls are not gated on queued x traffic
                def load_wqk(m):
                    wm = wqp.tile([P, CT, P], F8, tag="wm", name=f"wm{m}")
                    nc.sync.dma_start(wm, d["wqk"][m])
                    return wm
                pre_wm = {m: load_wqk(m) for m in range(2)}

                def src_ln1(toff, tn):
                    xc = lnw.tile([P, CT, 512], F32, tag="ln_xc")
                    nc.sync.dma_start(xc[:, :, :tn], d["xt"][:, :, toff:toff + tn])
                    return xc[:, :, :tn]
                _layernorm(nc, lnw, psln1, src_ln1, N, ln1g_sb, ln1b_sb,
                           eps_sb, ones, hT,
                           chunk_list=[(0, 256), (256, 256), (512, 512),
                                       (1024, 346)])

                wv_sb = wvp.tile([P, CT, C], F8)
                nc.sync.dma_start(wv_sb, d["wv"][:, :, :])

                wms = {}

                def qk_mm(m, qoff, qn):
                    if m not in wms:
                        wms[m] = pre_wm.pop(m) if m in pre_wm else load_wqk(m)
                    dest = QTt if m < 8 else KTt
                    ps = psA.tile([P, 512], F32, tag="ps", name=f"ps{m}_{qoff}")
                    for k in range(CT // 2):
                        nc.tensor.matmul(ps[:, :qn],
                                         wms[m][:, 2 * k:2 * k + 2, :],
                                         hT[:, 2 * k:2 * k + 2, qoff:qoff + qn],
                                         start=(k == 0), stop=(k == CT // 2 - 1),
                                         perf_mode=mybir.MatmulPerfMode.DoubleRow)
                    s2 = (DH ** -0.5) / WS if m < 8 else 1.0 / WS
                    nc.vector.tensor_scalar(dest[:, m % 8, qoff:qoff + qn],
                                            ps[:, :qn],
                                            bqk_sb[:, m:m + 1], s2, ADD, MUL)

                def v_mm(t):
                    tp = min(P, N - t * P)
                    ps = psV.tile([P, 2, 512], F32, tag="psv", name=f"psv{t}")
                    for vc in range(2):
                        for k in range(CT // 2):
                            nc.tensor.matmul(ps[:tp, vc, :],
                                             hT[:, 2 * k:2 * k + 2, t * P:t * P + tp],
                                             wv_sb[:, 2 * k:2 * k + 2,
                                                   vc * 512:(vc + 1) * 512],
                                             start=(k == 0), stop=(k == CT // 2 - 1),
                                             perf_mode=mybir.MatmulPerfMode.DoubleRow)
                    # evac on ACT so AV matmuls depend on one engine only
                    # (b_v is folded into the proj bias on the host)
                    nc.scalar.mul(
                        vaug[:tp, t, :, :DH],
                        ps[:tp, :, :].rearrange("p v (h dh) -> p (v h) dh", dh=DH),
                        1.0 / WS)

                # wave 0: tokens [0,512) ready first
                for m in range(8):
                    qk_mm(m, 0, 512)
                for m in range(8, 16):
                    qk_mm(m, 0, 512)
                for t in range(4):
                    v_mm(t)
                # wave 1: tokens [512,1024)
                for m in range(8):
                    qk_mm(m, 512, Q - 512)
                for m in range(8, 16):
                    qk_mm(m, 512, 512)
                for t in range(4, 8):
                    v_mm(t)
                # wave 2: tokens [1024,1370)
                for m in range(8, 16):
                    qk_mm(m, 1024, N - 1024)
                for t in range(8, KT):
                    v_mm(t)

            # ---------- Phase C: attention ----------
            # (xq / wproj loads issued first so they overlap attention)
            xqp = tc.alloc_tile_pool(name="xqp", bufs=1)
            wfc1_sb = xqp.tile([P, CT, HID], F8)
            nc.sync.dma_start(wfc1_sb, d["wfc1"][:, :, :])
            prx = tc.alloc_tile_pool(name="prx", bufs=1)
            xq = prx.tile([P, CT, Q], F32)
            nc.sync.dma_start(xq, d["xt"][:, :, :Q])
            wproj_sb = prx.tile([P, CT, C], F8)
            nc.sync.dma_start(wproj_sb, d["wproj"][:, :, :])

            GROUPS = [[0, 1, 2], [3, 4, 5], [6, 7, 8], [9, 10]]
            with tc.tile_pool(name="pss", bufs=2, space="PSUM") as pss, \
                 tc.tile_pool(name="psav", bufs=2, space="PSUM") as psav, \
                 tc.tile_pool(name="ptp", bufs=3) as ptp, \
                 tc.tile_pool(name="nrmd", bufs=2, space="DRAM") as nrmd, \
                 tc.tile_pool(name="nrm", bufs=2) as nrm:
                work_items = [(h, qoff, qn) for h in range(H)
                              for (qoff, qn) in QCH]
                pending = []  # (pt, grp, h, qn, ps_av)

                def flush_av(pending):
                    pt, grp, h, qn, ps_av = pending.pop(0)
                    for jj, j in enumerate(grp):
                        tp = min(P, N - j * P)
                        nc.tensor.matmul(ps_av[:DH + 1, :qn],
                                         vaug[:tp, j, h, :],
                                         pt[:tp, jj, :qn],
                                         start=(j == 0), stop=(j == KT - 1))

                av_tiles = {}
                for wi, (h, qoff, qn) in enumerate(work_items):
                    ht_, hoff = h // 2, (h % 2) * DH
                    ps_av = psav.tile([P, 512], F32, tag="av", name=f"av{wi}")
                    av_tiles[wi] = (ps_av, h, ht_, hoff, qoff, qn)
                    for gi, grp in enumerate(GROUPS):
                        ps_s = pss.tile([P, 3, 512], F32, tag="s",
                                        name=f"s{wi}_{gi}")
                        if gi == len(GROUPS) - 1:
                            # pad the ragged tile's rows so one exp call
                            # covers the group (exp(-30)~=0); full partition
                            # range (PSUM wants 32-aligned offsets), the
                            # matmul below then overwrites rows [0, 90)
                            nc.vector.memset(ps_s[:, 1, :qn], -30.0)
                        for jj, j in enumerate(grp):
                            tp = min(P, N - j * P)
                            nc.tensor.matmul(
                                ps_s[:tp, jj, :qn],
                                KTt[hoff:hoff + DH, ht_, j * P:j * P + tp],
                                QTt[hoff:hoff + DH, ht_, qoff:qoff + qn],
                                start=True, stop=True)
                        pt = ptp.tile([P, 3, 512], BF16, tag="pt",
                                      name=f"pt{wi}_{gi}")
                        nc.scalar.activation(pt[:, :len(grp), :qn],
                                             ps_s[:, :len(grp), :qn], AF.Exp)
                        pending.append((pt, grp, h, qn, ps_av))
                        if len(pending) > 1:
                            flush_av(pending)
                    # normalize the item whose AV chain completed
                    done = wi - 1 if wi > 0 else None
                    if wi == len(work_items) - 1:
                        while pending:
                            flush_av(pending)
                        done_list = [wi - 1, wi] if wi > 0 else [wi]
                    elif done is not None:
                        done_list = [done]
                    else:
                        done_list = []
                    for dwi in done_list:
                        pav, dh_, dht, dhoff, dqoff, dqn = av_tiles.pop(dwi)
                        rrow = nrm.tile([1, 512], F32, tag="rrow",
                                        name=f"rr{dwi}")
                        nc.vector.reciprocal(rrow[:, :dqn],
                                             pav[DH:DH + 1, :dqn])
                        rdram = nrmd.tile([1, 512], F32, tag="rd",
                                          name=f"rd{dwi}")
                        nc.sync.dma_start(rdram[:, :dqn], rrow[:, :dqn])
                        rbc = nrm.tile([DH, 512], F32, tag="rbc",
                                       name=f"rb{dwi}")
                        nc.sync.dma_start(rbc[:, :dqn],
                                          _pbroadcast(rdram[:1, :dqn], DH))
                        nc.vector.tensor_tensor(
                            oT[dhoff:dhoff + DH, dht, dqoff:dqoff + dqn],
                            pav[:DH, :dqn], rbc[:, :dqn], MUL)

            # ---------- Phase D: proj + residual + LN2 ----------
            with tc.tile_pool(name="prw", bufs=2) as prw, \
                 tc.tile_pool(name="psln2", bufs=2, space="PSUM") as psln2, \
                 tc.tile_pool(name="pspr", bufs=4, space="PSUM") as pspr:
                # qc outer so x1T's first chunk completes early (LN2 can start)
                for (qoff, qn) in QCH:
                    for m in range(CT):
                        ps = pspr.tile([P, 512], F32, tag="ps")
                        for k in range(CT // 2):
                            nc.tensor.matmul(ps[:, :qn],
                                             wproj_sb[:, 2 * k:2 * k + 2,
                                                      m * P:(m + 1) * P],
                                             oT[:, 2 * k:2 * k + 2, qoff:qoff + qn],
                                             start=(k == 0), stop=(k == CT // 2 - 1),
                                             perf_mode=mybir.MatmulPerfMode.DoubleRow)
                        tmp = prw.tile([P, 512], F32, tag="prtmp")
                        nc.vector.tensor_scalar(tmp[:, :qn], ps[:, :qn],
                                                g1s_sb[:, m:m + 1],
                                                bproj_sb[:, m:m + 1], MUL, ADD)
                        nc.gpsimd.tensor_add(x1T[:, m, qoff:qoff + qn], tmp[:, :qn],
                                             xq[:, m, qoff:qoff + qn])
                _layernorm(nc, prw, psln2,
                           lambda toff, tn: x1T[:, :, toff:toff + tn],
                           Q, ln2g_sb, ln2b_sb, eps_sb, ones, h2T)
            prx.release()

            # ---------- Phase E: MLP + residual ----------
            with tc.tile_pool(name="f2w", bufs=2) as f2w, \
                 tc.tile_pool(name="gel", bufs=1) as gel, \
                 tc.tile_pool(name="outp", bufs=2) as outp, \
                 tc.tile_pool(name="psml", bufs=2, space="PSUM") as psml, \
                 tc.tile_pool(name="psm2", bufs=4, space="PSUM") as psm2:
                geluT = gel.tile([P, HT, 2, 512], F8)
                for m in range(HT):
                    ps = psml.tile([P, 2, 512], F32, tag="ps2", name=f"ps2_{m}")
                    for k in range(CT // 2):
                        for ci, (qoff, qn) in enumerate(QCH):
                            nc.tensor.matmul(ps[:, ci, :qn],
                                             wfc1_sb[:, 2 * k:2 * k + 2,
                                                     m * P:(m + 1) * P],
                                             h2T[:, 2 * k:2 * k + 2, qoff:qoff + qn],
                                             start=(k == 0), stop=(k == CT // 2 - 1),
                                             perf_mode=mybir.MatmulPerfMode.DoubleRow)
                    nc.vector.memset(ps[:, 1, QCH[1][1]:], 0.0)
                    nc.scalar.activation(geluT[:, m, :, :], ps[:, :, :], AF.Gelu,
                                         bias=bfc1_sb[:, m:m + 1], scale=1.0 / WS)
                for m in range(CT):
                    w2 = f2w.tile([P, HT, P], F8, tag="w2")
                    nc.sync.dma_start(w2, d["wfc2"][m])
                    om = outp.tile([P, Q], F32, tag="om")
                    pss_ = [psm2.tile([P, 512], F32, tag="ps", name=f"psml{ci}") for ci in range(len(QCH))]
                    for k in range(HT // 2):
                        for ci, (qoff, qn) in enumerate(QCH):
                            nc.tensor.matmul(pss_[ci][:, :qn],
                                             w2[:, 2 * k:2 * k + 2, :],
                                             geluT[:, 2 * k:2 * k + 2, ci, :qn],
                                             start=(k == 0), stop=(k == HT // 2 - 1),
                                             perf_mode=mybir.MatmulPerfMode.DoubleRow)
                    for ci, (qoff, qn) in enumerate(QCH):
                        tmp = outp.tile([P, 512], F32, tag="f2tmp",
                                        name=f"f2tmp{ci}")
                        nc.vector.tensor_scalar(tmp[:, :qn], pss_[ci][:, :qn],
                                                g2s_sb[:, m:m + 1],
                                                bfc2_sb[:, m:m + 1], MUL, ADD)
                        nc.gpsimd.tensor_add(om[:, qoff:qoff + qn], tmp[:, :qn],
                                             x1T[:, m, qoff:qoff + qn])
                    nc.sync.dma_start(out_d[:, m, :], om[:, :])
            xqp.release()

    _legalize_matmul_waits(nc)
    return nc


_PROGRAM = None


def _get_program():
    global _PROGRAM
    if _PROGRAM is None:
        _PROGRAM = _build_program()
    return _PROGRAM


def _ptile(w, n_out_tiles, n_in_tiles, dtype=None):
    """[Cin, Cout] -> [m, p, k, col] pretiled lhsT layout."""
    a = w.reshape(n_in_tiles, P, n_out_tiles, P)
    return np.ascontiguousarray(a.transpose(2, 1, 0, 3)).astype(
        dtype if dtype is not None else ml_dtypes.bfloat16)


def _col_layout(v):
    """[D] -> [P, D//P] with column j = dims j*128..j*128+127."""
    return np.ascontiguousarray(v.reshape(-1, P).T).astype(np.float32)


def prepare_inputs(x, ln1_g, ln1_b, w_qkv, b_qkv, w_proj, b_proj, gamma1,
                   ln2_g, ln2_b, w_fc1, b_fc1, w_fc2, b_fc2, gamma2):
    """Host-side prep: returns (shared weight map, per-core input maps)."""
    wqkvT = np.ascontiguousarray(w_qkv.T).astype(np.float32)  # [C, 3C]
    b_qkv = np.asarray(b_qkv, np.float32)
    gamma1 = np.asarray(gamma1, np.float32)
    gamma2 = np.asarray(gamma2, np.float32)

    # fp8 weights are stored scaled by WS (unscaled at PSUM evacuation);
    # the attention 1/sqrt(dh) and the layer-scale gammas are applied at
    # evacuation time too (folding them here would denormalize e4m3)
    wm = {}
    wm["wqk"] = _ptile(wqkvT[:, :2 * C] * WS, 16, CT, F8NP)
    wm["bqk"] = _col_layout(b_qkv[:2 * C] * WS)
    wv = np.ascontiguousarray(wqkvT[:, 2 * C:])  # [C, C]
    wm["wv"] = np.ascontiguousarray(
        (wv * WS).reshape(CT, P, C).transpose(1, 0, 2)).astype(F8NP)
    wprojT = np.asarray(w_proj, np.float32).T
    wm["wproj"] = np.ascontiguousarray(
        (wprojT * WS).reshape(CT, P, C).transpose(1, 0, 2)).astype(F8NP)
    # b_v passes through softmax unchanged (convex combination), fold it here
    b_v = b_qkv[2 * C:]
    bproj_eff = (np.asarray(b_proj, np.float32)
                 + b_v @ np.asarray(w_proj, np.float32).T)
    wm["bproj"] = _col_layout(bproj_eff * gamma1)
    wm["g1s"] = _col_layout(gamma1 / WS)
    wm["g2s"] = _col_layout(gamma2 / WS)
    wm["ln1g"] = _col_layout(np.asarray(ln1_g, np.float32))
    wm["ln1b"] = _col_layout(np.asarray(ln1_b, np.float32))
    wm["ln2g"] = _col_layout(np.asarray(ln2_g, np.float32))
    wm["ln2b"] = _col_layout(np.asarray(ln2_b, np.float32))
    wfc1T = np.asarray(w_fc1, np.float32).T * WS
    wm["wfc1"] = np.ascontiguousarray(
        wfc1T.reshape(CT, P, HID).transpose(1, 0, 2)).astype(F8NP)
    wm["bfc1"] = _col_layout(np.asarray(b_fc1, np.float32))
    wm["wfc2"] = _ptile(np.asarray(w_fc2, np.float32).T * WS, CT, HT, F8NP)
    wm["bfc2"] = _col_layout(np.asarray(b_fc2, np.float32) * gamma2)

    in_maps = []
    x = np.asarray(x, np.float32)
    for core in range(NCORES):
        b, t = core // 2, core % 2
        xb = np.roll(x[b], -t * Q, axis=0)  # queries become tokens [0, Q)
        xtl = np.ascontiguousarray(
            xb.T.reshape(CT, P, N).transpose(1, 0, 2)).astype(np.float32)
        m = dict(wm)
        m["xt"] = xtl
        in_maps.append(m)
    return in_maps


def gather_output(results):
    out = np.empty((B, N, C), np.float32)
    for core in range(NCORES):
        b, t = core // 2, core % 2
        o = results[core]["out"]  # [P, CT, Q]
        out[b, t * Q:(t + 1) * Q, :] = o.transpose(1, 0, 2).reshape(C, Q).T
    return out


def kernel(**inputs):
    nc = _get_program()
    in_maps = prepare_inputs(**{k: np.asarray(v) for k, v in inputs.items()})
    res = run_bass_kernel_spmd(nc, in_maps, list(range(NCORES)))
    return gather_output(res.results)


if __name__ == "__main__":
    _get_program()
    print("program built OK")


# revision 58
# speedup vs baseline: 300.9098x; 1.0295x over previous
"""Trainium2 Bass kernel for a ViT-style transformer block (B=4, N=1370, C=1024).

Sharding: 8 cores = 4 batches x 2 token-halves. Each core runs the full block
for its 685 query tokens; K/V are computed for all 1370 tokens of its batch
(no collectives needed). The token-half selection is done by rolling the token
axis on the host so every core runs an identical program on tokens [0, 685).

On-chip layout: activations are kept feature-on-partition ("transposed",
[C, tokens]) the whole way through:
  - layernorm stats (sum, sum of squares over C) via ones-matmul on the PE,
    with lhsT = ones[128,128] so the stats are partition-broadcast for free
  - per-channel affines (ln gamma/beta, biases, layer-scale gammas) are
    per-partition scalars (native tensor_scalar broadcast)
  - attention computes S^T = K @ Q^T per head; exp on ScalarE directly from
    PSUM; A@V is lhsT=[V|ones] so the softmax denominator rides along as one
    extra output row; normalization via reciprocal + partition-broadcast DMA
All projection GEMMs (QKV, attn-out, fc1, fc2) run in fp8e4m3 with DoubleRow
perf mode (weights scaled x256 on the host — 0.02-scale weights would be
denormal in e4m3 — and unscaled during PSUM evacuation); the attention core
(Q^T/K^T/V/P) is bf16 and the residual stream stays fp32. The 1e-5
layer-scale makes the branch contributions tiny relative to the fp32
pass-through of x, so overall output error stays ~3e-7 relative.
The emission order follows data readiness (QKV in token-chunk "waves",
software-pipelined scores->exp->AV across heads) because engine streams
execute in order. A post-scheduling pass legalizes multi-wait instructions
for this walrus build (one sync wait per instruction).
"""

import numpy as np
import ml_dtypes

import concourse.bass as bass
import concourse.mybir as mybir
import concourse.tile as tile
from concourse.bass_utils import run_bass_kernel_spmd

B, N, C = 4, 1370, 1024
H, DH, HID = 16, 64, 4096
P = 128
CT = C // P            # 8 feature tiles
HT = HID // P          # 32 hidden tiles
NCORES = 8
Q = N // 2             # 685 query tokens per core
KT = (N + P - 1) // P  # 11 key-token tiles (last has 90 rows)
EPS = 1e-5

F32 = mybir.dt.float32
BF16 = mybir.dt.bfloat16
F8 = mybir.dt.float8e4
F8NP = mybir.dt.np(F8)
WS = 256.0           # fp8 weight scale (0.02-scale weights are denormal in e4m3)
NP = 1376            # N padded to a 16 multiple (fp8 DoubleRow stride rule)
QP = 688             # Q padded likewise
ADD = mybir.AluOpType.add
SUB = mybir.AluOpType.subtract
MUL = mybir.AluOpType.mult
AF = mybir.ActivationFunctionType


def _chunks(total, size):
    out = []
    off = 0
    while off < total:
        out.append((off, min(size, total - off)))
        off += size
    return out


QCH = _chunks(Q, 512)   # query-token chunks
TCH = _chunks(N, 512)   # full-token chunks
QCM = [(0, 343), (343, 342)]  # balanced MLP chunks (less gelu padding)


def _pbroadcast(ap, n):
    """Partition-broadcast an AP whose partition dim is 1 to n partitions."""
    return bass.AP(tensor=ap.tensor, offset=ap.offset, ap=[[0, n]] + list(ap.ap[1:]))


def _layernorm(nc, work, psum, src_of, ntok, g_sb, b_sb, eps_sb, ones, out_ht,
               csz=512, chunk_list=None):
    """LN over the feature axis (partitions). src_of(off, n) -> fp32 AP [P, CT, n].
    Writes normalized output into out_ht[:, k, off:off+n]."""
    chunks = chunk_list if chunk_list is not None else _chunks(ntok, csz)
    for (toff, tn) in chunks:
        xc = src_of(toff, tn)
        ps_sx = psum.tile([P, 512], F32, tag="ps")
        ps_sx2 = psum.tile([P, 512], F32, tag="ps")
        for k in range(CT):
            xb = work.tile([P, csz], BF16, tag="ln_xb")
            nc.gpsimd.tensor_copy(xb[:, :tn], xc[:, k, :])
            x2 = work.tile([P, csz], BF16, tag="ln_x2")
            nc.scalar.activation(x2[:, :tn], xc[:, k, :], AF.Square)
            nc.tensor.matmul(ps_sx[:, :tn], ones, xb[:, :tn],
                             start=(k == 0), stop=(k == CT - 1))
            nc.tensor.matmul(ps_sx2[:, :tn], ones, x2[:, :tn],
                             start=(k == 0), stop=(k == CT - 1))
        mean = work.tile([P, csz], F32, tag="ln_mean")
        nc.vector.tensor_scalar_mul(mean[:, :tn], ps_sx[:, :tn], 1.0 / C)
        rstd = work.tile([P, csz], F32, tag="ln_rstd")
        nc.vector.tensor_mul(rstd[:, :tn], mean[:, :tn], mean[:, :tn])
        nc.vector.scalar_tensor_tensor(rstd[:, :tn], ps_sx2[:, :tn], 1.0 / C,
                                       rstd[:, :tn], MUL, SUB)
        nc.scalar.activation(rstd[:, :tn], rstd[:, :tn], AF.Sqrt,
                             bias=eps_sb, scale=1.0)
        nc.vector.reciprocal(rstd[:, :tn], rstd[:, :tn])
        for k in range(CT):
            xm = work.tile([P, csz], F32, tag="ln_xm")
            nc.vector.tensor_tensor(xm[:, :tn], xc[:, k, :], mean[:, :tn], SUB)
            nc.vector.scalar_tensor_tensor(xm[:, :tn], xm[:, :tn],
                                           g_sb[:, k:k + 1], rstd[:, :tn],
                                           MUL, MUL)
            nc.vector.tensor_scalar_add(out_ht[:, k, toff:toff + tn],
                                        xm[:, :tn], b_sb[:, k:k + 1])


_WAIT_EXEMPT = {
    "InstEventSemaphore", "InstNoOp",
    "InstCall", "InstBranchHint", "InstHalt", "InstCollectiveCompute",
}


def _legalize_matmul_waits(nc):
    """This walrus build allows only ONE sync wait per compute instruction.
    Move extra waits onto NoOps inserted immediately before the instruction
    (same engine stream position => identical ordering semantics)."""
    nid = [0]
    for fn in nc.m.functions:
        for blk in fn.blocks:
            insts = blk.instructions
            i = 0
            while i < len(insts):
                ins = insts[i]
                tname = type(ins).__name__
                si = getattr(ins, "sync_info", None)
                if (tname not in _WAIT_EXEMPT and tname.startswith("Inst")
                        and si is not None and len(si.on_wait) > 1):
                    waits = list(si.on_wait)
                    for w in waits[:-1]:
                        nop = mybir.InstNoOp(
                            name=f"I-mmwait-{nid[0]}", engine=ins.engine,
                            ins=[], outs=[],
                            sync_info=mybir.SyncInfo(on_wait=[w],
                                                     on_update=[]))
                        nid[0] += 1
                        insts.insert(i, nop)
                        i += 1
                    ins.sync_info = mybir.SyncInfo(on_wait=[waits[-1]],
                                                   on_update=si.on_update)
                i += 1


def _build_program():
    nc = bass.Bass()
    d = {}
    d["xt"] = nc.declare_dram_parameter("xt", [P, CT, N], F32, isOutput=False)
    d["wqk"] = nc.declare_dram_parameter("wqk", [16, P, CT, P], F8, isOutput=False)
    d["bqk"] = nc.declare_dram_parameter("bqk", [P, 16], F32, isOutput=False)
    d["wv"] = nc.declare_dram_parameter("wv", [P, CT, C], F8, isOutput=False)
    d["wproj"] = nc.declare_dram_parameter("wproj", [P, CT, C], F8, isOutput=False)
    d["g1s"] = nc.declare_dram_parameter("g1s", [P, CT], F32, isOutput=False)
    d["g2s"] = nc.declare_dram_parameter("g2s", [P, CT], F32, isOutput=False)
    d["bproj"] = nc.declare_dram_parameter("bproj", [P, CT], F32, isOutput=False)
    d["ln1g"] = nc.declare_dram_parameter("ln1g", [P, CT], F32, isOutput=False)
    d["ln1b"] = nc.declare_dram_parameter("ln1b", [P, CT], F32, isOutput=False)
    d["ln2g"] = nc.declare_dram_parameter("ln2g", [P, CT], F32, isOutput=False)
    d["ln2b"] = nc.declare_dram_parameter("ln2b", [P, CT], F32, isOutput=False)
    d["wfc1"] = nc.declare_dram_parameter("wfc1", [P, CT, HID], F8, isOutput=False)
    d["bfc1"] = nc.declare_dram_parameter("bfc1", [P, HT], F32, isOutput=False)
    d["wfc2"] = nc.declare_dram_parameter("wfc2", [CT, P, HT, P], F8, isOutput=False)
    d["bfc2"] = nc.declare_dram_parameter("bfc2", [P, CT], F32, isOutput=False)
    out_d = nc.declare_dram_parameter("out", [P, CT, Q], F32, isOutput=True)

    with tile.TileContext(nc) as tc:
        with tc.tile_pool(name="const", bufs=1) as const, \
             tc.tile_pool(name="persist", bufs=1) as persist:
            ones = const.tile([P, P], BF16)
            nc.vector.memset(ones, 1.0)
            eps_sb = const.tile([P, 1], F32)
            nc.vector.memset(eps_sb, EPS)

            def load_const(name, shape):
                t = const.tile(shape, F32, tag=f"const_{name}")
                nc.sync.dma_start(t, d[name][:, :])
                return t

            ln1g_sb = load_const("ln1g", [P, CT])
            ln1b_sb = load_const("ln1b", [P, CT])
            ln2g_sb = load_const("ln2g", [P, CT])
            ln2b_sb = load_const("ln2b", [P, CT])
            bqk_sb = load_const("bqk", [P, 16])
            bproj_sb = load_const("bproj", [P, CT])
            g1s_sb = load_const("g1s", [P, CT])
            g2s_sb = load_const("g2s", [P, CT])
            bfc1_sb = load_const("bfc1", [P, HT])
            bfc2_sb = load_const("bfc2", [P, CT])

            hT = persist.tile([P, CT, NP], F8)       # ln1 output, all tokens
            QTt = persist.tile([P, CT, Q], BF16)     # Q^T (scaled by dh^-0.5)
            KTt = persist.tile([P, CT, N], BF16)     # K^T
            vaug = persist.tile([P, KT, H, DH + 1], BF16)  # V | ones, token-partition
            oT = persist.tile([P, CT, QP], F8)       # attention out, normalized
            x1T = persist.tile([P, CT, Q], F32)      # residual after attention
            h2T = persist.tile([P, CT, QP], F8)      # ln2 output

            nc.vector.memset(vaug[:, :, :, DH:DH + 1], 1.0)

            # warmup matmul so the PE clock observes the DVE memsets before
            # any data matmul (walrus allows only one sync wait per Matmult)
            with tc.tile_pool(name="warm", bufs=1, space="PSUM") as warm:
                wps = warm.tile([P, P], F32)
                nc.tensor.matmul(wps, ones, ones, start=True, stop=True)

            # ---------- Phase A+B: LN1 + QKV projections ----------
            with tc.tile_pool(name="lnw", bufs=2) as lnw, \
                 tc.tile_pool(name="wqp", bufs=16) as wqp, \
                 tc.tile_pool(name="wvp", bufs=1) as wvp, \
                 tc.tile_pool(name="psln1", bufs=2, space="PSUM") as psln1, \
                 tc.tile_pool(name="psA", bufs=2, space="PSUM") as psA, \
                 tc.tile_pool(name="psV", bufs=2, space="PSUM") as psV:
                # hoist weight DMAs ahead of the x-chunk DMAs so the first
                # QK matmuls are not gated on queued x traffic
                def load_wqk(m):
                    wm = wqp.tile([P, CT, P], F8, tag="wm", name=f"wm{m}")
                    nc.sync.dma_start(wm, d["wqk"][m])
                    return wm
                pre_wm = {m: load_wqk(m) for m in range(2)}

                def src_ln1(toff, tn):
                    xc = lnw.tile([P, CT, 512], F32, tag="ln_xc")
                    nc.sync.dma_start(xc[:, :, :tn], d["xt"][:, :, toff:toff + tn])
                    return xc[:, :, :tn]
                _layernorm(nc, lnw, psln1, src_ln1, N, ln1g_sb, ln1b_sb,
                           eps_sb, ones, hT,
                           chunk_list=[(0, 256), (256, 256), (512, 512),
                                       (1024, 346)])

                wv_sb = wvp.tile([P, CT, C], F8)
                nc.sync.dma_start(wv_sb, d["wv"][:, :, :])

                wms = {}

                def qk_mm(m, qoff, qn):
                    if m not in wms:
                        wms[m] = pre_wm.pop(m) if m in pre_wm else load_wqk(m)
                    dest = QTt if m < 8 else KTt
                    ps = psA.tile([P, 512], F32, tag="ps", name=f"ps{m}_{qoff}")
                    for k in range(CT // 2):
                        nc.tensor.matmul(ps[:, :qn],
                                         wms[m][:, 2 * k:2 * k + 2, :],
                                         hT[:, 2 * k:2 * k + 2, qoff:qoff + qn],
                                         start=(k == 0), stop=(k == CT // 2 - 1),
                                         perf_mode=mybir.MatmulPerfMode.DoubleRow)
                    s2 = (DH ** -0.5) / WS if m < 8 else 1.0 / WS
                    nc.vector.tensor_scalar(dest[:, m % 8, qoff:qoff + qn],
                                            ps[:, :qn],
                                            bqk_sb[:, m:m + 1], s2, ADD, MUL)

                def v_mm(t):
                    tp = min(P, N - t * P)
                    ps = psV.tile([P, 2, 512], F32, tag="psv", name=f"psv{t}")
                    for vc in range(2):
                        for k in range(CT // 2):
                            nc.tensor.matmul(ps[:tp, vc, :],
                                             hT[:, 2 * k:2 * k + 2, t * P:t * P + tp],
                                             wv_sb[:, 2 * k:2 * k + 2,
                                                   vc * 512:(vc + 1) * 512],
                                             start=(k == 0), stop=(k == CT // 2 - 1),
                                             perf_mode=mybir.MatmulPerfMode.DoubleRow)
                    # evac on ACT so AV matmuls depend on one engine only
                    # (b_v is folded into the proj bias on the host)
                    nc.scalar.mul(
                        vaug[:tp, t, :, :DH],
                        ps[:tp, :, :].rearrange("p v (h dh) -> p (v h) dh", dh=DH),
                        1.0 / WS)

                # wave 0: tokens [0,512) ready first
                for m in range(8):
                    qk_mm(m, 0, 512)
                for m in range(8, 16):
                    qk_mm(m, 0, 512)
                for t in range(4):
                    v_mm(t)
                # wave 1: tokens [512,1024)
                for m in range(8):
                    qk_mm(m, 512, Q - 512)
                for m in range(8, 16):
                    qk_mm(m, 512, 512)
                for t in range(4, 8):
                    v_mm(t)
                # wave 2: tokens [1024,1370)
                for m in range(8, 16):
                    qk_mm(m, 1024, N - 1024)
                for t in range(8, KT):
                    v_mm(t)

            # ---------- Phase C: attention ----------
            # (xq / wproj loads issued first so they overlap attention)
            xqp = tc.alloc_tile_pool(name="xqp", bufs=1)
            wfc1_sb = xqp.tile([P, CT, HID], F8)
            nc.sync.dma_start(wfc1_sb, d["wfc1"][:, :, :])
            prx = tc.alloc_tile_pool(name="prx", bufs=1)
            xq = prx.tile([P, CT, Q], F32)
            nc.sync.dma_start(xq, d["xt"][:, :, :Q])
            wproj_sb = prx.tile([P, CT, C], F8)
            nc.sync.dma_start(wproj_sb, d["wproj"][:, :, :])

            GROUPS = [[0, 1, 2], [3, 4, 5], [6, 7, 8], [9, 10]]
            with tc.tile_pool(name="pss", bufs=2, space="PSUM") as pss, \
                 tc.tile_pool(name="psav", bufs=2, space="PSUM") as psav, \
                 tc.tile_pool(name="ptp", bufs=3) as ptp, \
                 tc.tile_pool(name="nrmd", bufs=2, space="DRAM") as nrmd, \
                 tc.tile_pool(name="nrm", bufs=2) as nrm:
                work_items = [(h, qoff, qn) for h in range(H)
                              for (qoff, qn) in QCH]
                pending = []  # (pt, grp, h, qn, ps_av)

                def flush_av(pending):
                    pt, grp, h, qn, ps_av = pending.pop(0)
                    for jj, j in enumerate(grp):
                        tp = min(P, N - j * P)
                        nc.tensor.matmul(ps_av[:DH + 1, :qn],
                                         vaug[:tp, j, h, :],
                                         pt[:tp, jj, :qn],
                                         start=(j == 0), stop=(j == KT - 1))

                av_tiles = {}
                for wi, (h, qoff, qn) in enumerate(work_items):
                    ht_, hoff = h // 2, (h % 2) * DH
                    ps_av = psav.tile([P, 512], F32, tag="av", name=f"av{wi}")
                    av_tiles[wi] = (ps_av, h, ht_, hoff, qoff, qn)
                    for gi, grp in enumerate(GROUPS):
                        ps_s = pss.tile([P, 3, 512], F32, tag="s",
                                        name=f"s{wi}_{gi}")
                        if gi == len(GROUPS) - 1:
                            # pad the ragged tile's rows so one exp call
                            # covers the group (exp(-30)~=0); full partition
                            # range (PSUM wants 32-aligned offsets), the
                            # matmul below then overwrites rows [0, 90)
                            nc.vector.memset(ps_s[:, 1, :qn], -30.0)
                        for jj, j in enumerate(grp):
                            tp = min(P, N - j * P)
                            nc.tensor.matmul(
                                ps_s[:tp, jj, :qn],
                                KTt[hoff:hoff + DH, ht_, j * P:j * P + tp],
                                QTt[hoff:hoff + DH, ht_, qoff:qoff + qn],
                                start=True, stop=True)
                        pt = ptp.tile([P, 3, 512], BF16, tag="pt",
                                      name=f"pt{wi}_{gi}")
                        nc.scalar.activation(pt[:, :len(grp), :qn],
                                             ps_s[:, :len(grp), :qn], AF.Exp)
                        pending.append((pt, grp, h, qn, ps_av))
                        if len(pending) > 1:
                            flush_av(pending)
                    # normalize the item whose AV chain completed
                    done = wi - 1 if wi > 0 else None
                    if wi == len(work_items) - 1:
                        while pending:
                            flush_av(pending)
                        done_list = [wi - 1, wi] if wi > 0 else [wi]
                    elif done is not None:
                        done_list = [done]
                    else:
                        done_list = []
                    for dwi in done_list:
                        pav, dh_, dht, dhoff, dqoff, dqn = av_tiles.pop(dwi)
                        rrow = nrm.tile([1, 512], F32, tag="rrow",
                                        name=f"rr{dwi}")
                        nc.vector.reciprocal(rrow[:, :dqn],
                                             pav[DH:DH + 1, :dqn])
                        rdram = nrmd.tile([1, 512], F32, tag="rd",
                                          name=f"rd{dwi}")
                        nc.sync.dma_start(rdram[:, :dqn], rrow[:, :dqn])
                        rbc = nrm.tile([DH, 512], F32, tag="rbc",
                                       name=f"rb{dwi}")
                        nc.sync.dma_start(rbc[:, :dqn],
                                          _pbroadcast(rdram[:1, :dqn], DH))
                        nc.vector.tensor_tensor(
                            oT[dhoff:dhoff + DH, dht, dqoff:dqoff + dqn],
                            pav[:DH, :dqn], rbc[:, :dqn], MUL)

            # ---------- Phase D: proj + residual + LN2 ----------
            with tc.tile_pool(name="prw", bufs=2) as prw, \
                 tc.tile_pool(name="psln2", bufs=2, space="PSUM") as psln2, \
                 tc.tile_pool(name="pspr", bufs=4, space="PSUM") as pspr:
                # qc outer so x1T's first chunk completes early (LN2 can start)
                for (qoff, qn) in QCH:
                    for m in range(CT):
                        ps = pspr.tile([P, 512], F32, tag="ps")
                        for k in range(CT // 2):
                            nc.tensor.matmul(ps[:, :qn],
                                             wproj_sb[:, 2 * k:2 * k + 2,
                                                      m * P:(m + 1) * P],
                                             oT[:, 2 * k:2 * k + 2, qoff:qoff + qn],
                                             start=(k == 0), stop=(k == CT // 2 - 1),
                                             perf_mode=mybir.MatmulPerfMode.DoubleRow)
                        tmp = prw.tile([P, 512], F32, tag="prtmp")
                        nc.vector.tensor_scalar(tmp[:, :qn], ps[:, :qn],
                                                g1s_sb[:, m:m + 1],
                                                bproj_sb[:, m:m + 1], MUL, ADD)
                        nc.gpsimd.tensor_add(x1T[:, m, qoff:qoff + qn], tmp[:, :qn],
                                             xq[:, m, qoff:qoff + qn])
                _layernorm(nc, prw, psln2,
                           lambda toff, tn: x1T[:, :, toff:toff + tn],
                           Q, ln2g_sb, ln2b_sb, eps_sb, ones, h2T,
                           chunk_list=QCM)
            prx.release()

            # ---------- Phase E: MLP + residual ----------
            with tc.tile_pool(name="f2w", bufs=2) as f2w, \
                 tc.tile_pool(name="gel", bufs=1) as gel, \
                 tc.tile_pool(name="outp", bufs=2) as outp, \
                 tc.tile_pool(name="psml", bufs=2, space="PSUM") as psml, \
                 tc.tile_pool(name="psm2", bufs=4, space="PSUM") as psm2:
                geluT = gel.tile([P, HT, 2, 352], F8)
                for m in range(HT):
                    ps = psml.tile([P, 2, 512], F32, tag="ps2", name=f"ps2_{m}")
                    for k in range(CT // 2):
                        for ci, (qoff, qn) in enumerate(QCM):
                            nc.tensor.matmul(ps[:, ci, :qn],
                                             wfc1_sb[:, 2 * k:2 * k + 2,
                                                     m * P:(m + 1) * P],
                                             h2T[:, 2 * k:2 * k + 2, qoff:qoff + qn],
                                             start=(k == 0), stop=(k == CT // 2 - 1),
                                             perf_mode=mybir.MatmulPerfMode.DoubleRow)
                    nc.vector.memset(ps[:, 1, QCM[1][1]:], 0.0)
                    nc.scalar.activation(geluT[:, m, :, :343],
                                         ps[:, :, :343], AF.Gelu,
                                         bias=bfc1_sb[:, m:m + 1], scale=1.0 / WS)
                for m in range(CT):
                    w2 = f2w.tile([P, HT, P], F8, tag="w2")
                    nc.sync.dma_start(w2, d["wfc2"][m])
                    om = outp.tile([P, Q], F32, tag="om")
                    pss_ = [psm2.tile([P, 512], F32, tag="ps", name=f"psml{ci}") for ci in range(len(QCH))]
                    for k in range(HT // 2):
                        for ci, (qoff, qn) in enumerate(QCM):
                            nc.tensor.matmul(pss_[ci][:, :qn],
                                             w2[:, 2 * k:2 * k + 2, :],
                                             geluT[:, 2 * k:2 * k + 2, ci, :qn],
                                             start=(k == 0), stop=(k == HT // 2 - 1),
                                             perf_mode=mybir.MatmulPerfMode.DoubleRow)
                    for ci, (qoff, qn) in enumerate(QCM):
                        tmp = outp.tile([P, 512], F32, tag="f2tmp",
                                        name=f"f2tmp{ci}")
                        nc.vector.tensor_scalar(tmp[:, :qn], pss_[ci][:, :qn],
                                                g2s_sb[:, m:m + 1],
                                                bfc2_sb[:, m:m + 1], MUL, ADD)
                        nc.gpsimd.tensor_add(om[:, qoff:qoff + qn], tmp[:, :qn],
                                             x1T[:, m, qoff:qoff + qn])
                    nc.sync.dma_start(out_d[:, m, :], om[:, :])
            xqp.release()

    _legalize_matmul_waits(nc)
    return nc


_PROGRAM = None


def _get_program():
    global _PROGRAM
    if _PROGRAM is None:
        _PROGRAM = _build_program()
    return _PROGRAM


def _ptile(w, n_out_tiles, n_in_tiles, dtype=None):
    """[Cin, Cout] -> [m, p, k, col] pretiled lhsT layout."""
    a = w.reshape(n_in_tiles, P, n_out_tiles, P)
    return np.ascontiguousarray(a.transpose(2, 1, 0, 3)).astype(
        dtype if dtype is not None else ml_dtypes.bfloat16)


def _col_layout(v):
    """[D] -> [P, D//P] with column j = dims j*128..j*128+127."""
    return np.ascontiguousarray(v.reshape(-1, P).T).astype(np.float32)


def prepare_inputs(x, ln1_g, ln1_b, w_qkv, b_qkv, w_proj, b_proj, gamma1,
                   ln2_g, ln2_b, w_fc1, b_fc1, w_fc2, b_fc2, gamma2):
    """Host-side prep: returns (shared weight map, per-core input maps)."""
    wqkvT = np.ascontiguousarray(w_qkv.T).astype(np.float32)  # [C, 3C]
    b_qkv = np.asarray(b_qkv, np.float32)
    gamma1 = np.asarray(gamma1, np.float32)
    gamma2 = np.asarray(gamma2, np.float32)

    # fp8 weights are stored scaled by WS (unscaled at PSUM evacuation);
    # the attention 1/sqrt(dh) and the layer-scale gammas are applied at
    # evacuation time too (folding them here would denormalize e4m3)
    wm = {}
    wm["wqk"] = _ptile(wqkvT[:, :2 * C] * WS, 16, CT, F8NP)
    wm["bqk"] = _col_layout(b_qkv[:2 * C] * WS)
    wv = np.ascontiguousarray(wqkvT[:, 2 * C:])  # [C, C]
    wm["wv"] = np.ascontiguousarray(
        (wv * WS).reshape(CT, P, C).transpose(1, 0, 2)).astype(F8NP)
    wprojT = np.asarray(w_proj, np.float32).T
    wm["wproj"] = np.ascontiguousarray(
        (wprojT * WS).reshape(CT, P, C).transpose(1, 0, 2)).astype(F8NP)
    # b_v passes through softmax unchanged (convex combination), fold it here
    b_v = b_qkv[2 * C:]
    bproj_eff = (np.asarray(b_proj, np.float32)
                 + b_v @ np.asarray(w_proj, np.float32).T)
    wm["bproj"] = _col_layout(bproj_eff * gamma1)
    wm["g1s"] = _col_layout(gamma1 / WS)
    wm["g2s"] = _col_layout(gamma2 / WS)
    wm["ln1g"] = _col_layout(np.asarray(ln1_g, np.float32))
    wm["ln1b"] = _col_layout(np.asarray(ln1_b, np.float32))
    wm["ln2g"] = _col_layout(np.asarray(ln2_g, np.float32))
    wm["ln2b"] = _col_layout(np.asarray(ln2_b, np.float32))
    wfc1T = np.asarray(w_fc1, np.float32).T * WS
    wm["wfc1"] = np.ascontiguousarray(
        wfc1T.reshape(CT, P, HID).transpose(1, 0, 2)).astype(F8NP)
    wm["bfc1"] = _col_layout(np.asarray(b_fc1, np.float32))
    wm["wfc2"] = _ptile(np.asarray(w_fc2, np.float32).T * WS, CT, HT, F8NP)
    wm["bfc2"] = _col_layout(np.asarray(b_fc2, np.float32) * gamma2)

    in_maps = []
    x = np.asarray(x, np.float32)
    for core in range(NCORES):
        b, t = core // 2, core % 2
        xb = np.roll(x[b], -t * Q, axis=0)  # queries become tokens [0, Q)
        xtl = np.ascontiguousarray(
            xb.T.reshape(CT, P, N).transpose(1, 0, 2)).astype(np.float32)
        m = dict(wm)
        m["xt"] = xtl
        in_maps.append(m)
    return in_maps


def gather_output(results):
    out = np.empty((B, N, C), np.float32)
    for core in range(NCORES):
        b, t = core // 2, core % 2
        o = results[core]["out"]  # [P, CT, Q]
        out[b, t * Q:(t + 1) * Q, :] = o.transpose(1, 0, 2).reshape(C, Q).T
    return out


def kernel(**inputs):
    nc = _get_program()
    in_maps = prepare_inputs(**{k: np.asarray(v) for k, v in inputs.items()})
    res = run_bass_kernel_spmd(nc, in_maps, list(range(NCORES)))
    return gather_output(res.results)


if __name__ == "__main__":
    _get_program()
    print("program built OK")


# revision 61
# speedup vs baseline: 303.8836x; 1.0099x over previous
"""Trainium2 Bass kernel for a ViT-style transformer block (B=4, N=1370, C=1024).

Sharding: 8 cores = 4 batches x 2 token-halves. Each core runs the full block
for its 685 query tokens; K/V are computed for all 1370 tokens of its batch
(no collectives needed). The token-half selection is done by rolling the token
axis on the host so every core runs an identical program on tokens [0, 685).

On-chip layout: activations are kept feature-on-partition ("transposed",
[C, tokens]) the whole way through:
  - layernorm stats (sum, sum of squares over C) via ones-matmul on the PE,
    with lhsT = ones[128,128] so the stats are partition-broadcast for free
  - per-channel affines (ln gamma/beta, biases, layer-scale gammas) are
    per-partition scalars (native tensor_scalar broadcast)
  - attention computes S^T = K @ Q^T per head; exp on ScalarE directly from
    PSUM; A@V is lhsT=[V|ones] so the softmax denominator rides along as one
    extra output row; normalization via reciprocal + partition-broadcast DMA
All projection GEMMs (QKV, attn-out, fc1, fc2) run in fp8e4m3 with DoubleRow
perf mode (weights scaled x256 on the host — 0.02-scale weights would be
denormal in e4m3 — and unscaled during PSUM evacuation); the attention core
(Q^T/K^T/V/P) is bf16 and the residual stream stays fp32. The 1e-5
layer-scale makes the branch contributions tiny relative to the fp32
pass-through of x, so overall output error stays ~3e-7 relative.
The emission order follows data readiness (QKV in token-chunk "waves",
software-pipelined scores->exp->AV across heads) because engine streams
execute in order. A post-scheduling pass legalizes multi-wait instructions
for this walrus build (one sync wait per instruction).
"""

import numpy as np
import ml_dtypes

import concourse.bass as bass
import concourse.mybir as mybir
import concourse.tile as tile
from concourse.bass_utils import run_bass_kernel_spmd

B, N, C = 4, 1370, 1024
H, DH, HID = 16, 64, 4096
P = 128
CT = C // P            # 8 feature tiles
HT = HID // P          # 32 hidden tiles
NCORES = 8
Q = N // 2             # 685 query tokens per core
KT = (N + P - 1) // P  # 11 key-token tiles (last has 90 rows)
EPS = 1e-5

F32 = mybir.dt.float32
BF16 = mybir.dt.bfloat16
F8 = mybir.dt.float8e4
F8NP = mybir.dt.np(F8)
WS = 256.0           # fp8 weight scale (0.02-scale weights are denormal in e4m3)
NP = 1376            # N padded to a 16 multiple (fp8 DoubleRow stride rule)
QP = 688             # Q padded likewise
ADD = mybir.AluOpType.add
SUB = mybir.AluOpType.subtract
MUL = mybir.AluOpType.mult
AF = mybir.ActivationFunctionType


def _chunks(total, size):
    out = []
    off = 0
    while off < total:
        out.append((off, min(size, total - off)))
        off += size
    return out


QCH = _chunks(Q, 512)   # query-token chunks
TCH = _chunks(N, 512)   # full-token chunks
QCM = [(0, 343), (343, 342)]  # balanced MLP chunks (less gelu padding)


def _pbroadcast(ap, n):
    """Partition-broadcast an AP whose partition dim is 1 to n partitions."""
    return bass.AP(tensor=ap.tensor, offset=ap.offset, ap=[[0, n]] + list(ap.ap[1:]))


def _layernorm(nc, work, psum, src_of, ntok, g_sb, b_sb, eps_sb, ones, out_ht,
               csz=512, chunk_list=None):
    """LN over the feature axis (partitions). src_of(off, n) -> fp32 AP [P, CT, n].
    Writes normalized output into out_ht[:, k, off:off+n]."""
    chunks = chunk_list if chunk_list is not None else _chunks(ntok, csz)
    for (toff, tn) in chunks:
        xc = src_of(toff, tn)
        ps_sx = psum.tile([P, 512], F32, tag="ps")
        ps_sx2 = psum.tile([P, 512], F32, tag="ps")
        for k in range(CT):
            xb = work.tile([P, csz], BF16, tag="ln_xb")
            nc.gpsimd.tensor_copy(xb[:, :tn], xc[:, k, :])
            x2 = work.tile([P, csz], BF16, tag="ln_x2")
            nc.scalar.activation(x2[:, :tn], xc[:, k, :], AF.Square)
            nc.tensor.matmul(ps_sx[:, :tn], ones, xb[:, :tn],
                             start=(k == 0), stop=(k == CT - 1))
            nc.tensor.matmul(ps_sx2[:, :tn], ones, x2[:, :tn],
                             start=(k == 0), stop=(k == CT - 1))
        mean = work.tile([P, csz], F32, tag="ln_mean")
        nc.vector.tensor_scalar_mul(mean[:, :tn], ps_sx[:, :tn], 1.0 / C)
        rstd = work.tile([P, csz], F32, tag="ln_rstd")
        nc.vector.tensor_mul(rstd[:, :tn], mean[:, :tn], mean[:, :tn])
        nc.vector.scalar_tensor_tensor(rstd[:, :tn], ps_sx2[:, :tn], 1.0 / C,
                                       rstd[:, :tn], MUL, SUB)
        nc.scalar.activation(rstd[:, :tn], rstd[:, :tn], AF.Sqrt,
                             bias=eps_sb, scale=1.0)
        nc.vector.reciprocal(rstd[:, :tn], rstd[:, :tn])
        for k in range(CT):
            xm = work.tile([P, csz], F32, tag="ln_xm")
            nc.vector.tensor_tensor(xm[:, :tn], xc[:, k, :], mean[:, :tn], SUB)
            nc.vector.scalar_tensor_tensor(xm[:, :tn], xm[:, :tn],
                                           g_sb[:, k:k + 1], rstd[:, :tn],
                                           MUL, MUL)
            nc.vector.tensor_scalar_add(out_ht[:, k, toff:toff + tn],
                                        xm[:, :tn], b_sb[:, k:k + 1])


_WAIT_EXEMPT = {
    "InstEventSemaphore", "InstNoOp",
    "InstCall", "InstBranchHint", "InstHalt", "InstCollectiveCompute",
}


def _legalize_matmul_waits(nc):
    """This walrus build allows only ONE sync wait per compute instruction.
    Move extra waits onto NoOps inserted immediately before the instruction
    (same engine stream position => identical ordering semantics)."""
    nid = [0]
    for fn in nc.m.functions:
        for blk in fn.blocks:
            insts = blk.instructions
            i = 0
            while i < len(insts):
                ins = insts[i]
                tname = type(ins).__name__
                si = getattr(ins, "sync_info", None)
                if (tname not in _WAIT_EXEMPT and tname.startswith("Inst")
                        and si is not None and len(si.on_wait) > 1):
                    waits = list(si.on_wait)
                    for w in waits[:-1]:
                        nop = mybir.InstNoOp(
                            name=f"I-mmwait-{nid[0]}", engine=ins.engine,
                            ins=[], outs=[],
                            sync_info=mybir.SyncInfo(on_wait=[w],
                                                     on_update=[]))
                        nid[0] += 1
                        insts.insert(i, nop)
                        i += 1
                    ins.sync_info = mybir.SyncInfo(on_wait=[waits[-1]],
                                                   on_update=si.on_update)
                i += 1


def _build_program():
    nc = bass.Bass()
    d = {}
    d["xt"] = nc.declare_dram_parameter("xt", [P, CT, N], F32, isOutput=False)
    d["wqk"] = nc.declare_dram_parameter("wqk", [16, P, CT, P], F8, isOutput=False)
    d["bqk"] = nc.declare_dram_parameter("bqk", [P, 16], F32, isOutput=False)
    d["wv"] = nc.declare_dram_parameter("wv", [P, CT, C], F8, isOutput=False)
    d["wproj"] = nc.declare_dram_parameter("wproj", [P, CT, C], F8, isOutput=False)
    d["g1s"] = nc.declare_dram_parameter("g1s", [P, CT], F32, isOutput=False)
    d["g2s"] = nc.declare_dram_parameter("g2s", [P, CT], F32, isOutput=False)
    d["bproj"] = nc.declare_dram_parameter("bproj", [P, CT], F32, isOutput=False)
    d["ln1g"] = nc.declare_dram_parameter("ln1g", [P, CT], F32, isOutput=False)
    d["ln1b"] = nc.declare_dram_parameter("ln1b", [P, CT], F32, isOutput=False)
    d["ln2g"] = nc.declare_dram_parameter("ln2g", [P, CT], F32, isOutput=False)
    d["ln2b"] = nc.declare_dram_parameter("ln2b", [P, CT], F32, isOutput=False)
    d["wfc1"] = nc.declare_dram_parameter("wfc1", [P, CT, HID], F8, isOutput=False)
    d["bfc1"] = nc.declare_dram_parameter("bfc1", [P, HT], F32, isOutput=False)
    d["wfc2"] = nc.declare_dram_parameter("wfc2", [CT, P, HT, P], F8, isOutput=False)
    d["bfc2"] = nc.declare_dram_parameter("bfc2", [P, CT], F32, isOutput=False)
    out_d = nc.declare_dram_parameter("out", [P, CT, Q], F32, isOutput=True)

    with tile.TileContext(nc) as tc:
        with tc.tile_pool(name="const", bufs=1) as const, \
             tc.tile_pool(name="persist", bufs=1) as persist:
            ones = const.tile([P, P], BF16)
            nc.vector.memset(ones, 1.0)
            eps_sb = const.tile([P, 1], F32)
            nc.vector.memset(eps_sb, EPS)

            def load_const(name, shape):
                t = const.tile(shape, F32, tag=f"const_{name}")
                nc.sync.dma_start(t, d[name][:, :])
                return t

            ln1g_sb = load_const("ln1g", [P, CT])
            ln1b_sb = load_const("ln1b", [P, CT])
            ln2g_sb = load_const("ln2g", [P, CT])
            ln2b_sb = load_const("ln2b", [P, CT])
            bqk_sb = load_const("bqk", [P, 16])
            bproj_sb = load_const("bproj", [P, CT])
            g1s_sb = load_const("g1s", [P, CT])
            g2s_sb = load_const("g2s", [P, CT])
            bfc1_sb = load_const("bfc1", [P, HT])
            bfc2_sb = load_const("bfc2", [P, CT])

            hT = persist.tile([P, CT, NP], F8)       # ln1 output, all tokens
            QTt = persist.tile([P, CT, Q], BF16)     # Q^T (scaled by dh^-0.5)
            KTt = persist.tile([P, CT, N], BF16)     # K^T
            vaug = persist.tile([P, KT, H, DH + 1], BF16)  # V | ones, token-partition
            oT = persist.tile([P, CT, QP], F8)       # attention out, normalized
            x1T = persist.tile([P, CT, Q], F32)      # residual after attention
            h2T = persist.tile([P, CT, QP], F8)      # ln2 output

            nc.vector.memset(vaug[:, :, :, DH:DH + 1], 1.0)

            # warmup matmul so the PE clock observes the DVE memsets before
            # any data matmul (walrus allows only one sync wait per Matmult)
            with tc.tile_pool(name="warm", bufs=1, space="PSUM") as warm:
                wps = warm.tile([P, P], F32)
                nc.tensor.matmul(wps, ones, ones, start=True, stop=True)

            # ---------- Phase A+B: LN1 + QKV projections ----------
            with tc.tile_pool(name="lnw", bufs=2) as lnw, \
                 tc.tile_pool(name="wqp", bufs=16) as wqp, \
                 tc.tile_pool(name="wvp", bufs=1) as wvp, \
                 tc.tile_pool(name="psln1", bufs=2, space="PSUM") as psln1, \
                 tc.tile_pool(name="psA", bufs=2, space="PSUM") as psA, \
                 tc.tile_pool(name="psV", bufs=2, space="PSUM") as psV:
                # hoist weight DMAs ahead of the x-chunk DMAs so the first
                # QK matmuls are not gated on queued x traffic
                def load_wqk(m):
                    wm = wqp.tile([P, CT, P], F8, tag="wm", name=f"wm{m}")
                    nc.sync.dma_start(wm, d["wqk"][m])
                    return wm
                pre_wm = {m: load_wqk(m) for m in range(2)}

                def src_ln1(toff, tn):
                    xc = lnw.tile([P, CT, 512], F32, tag="ln_xc")
                    nc.sync.dma_start(xc[:, :, :tn], d["xt"][:, :, toff:toff + tn])
                    return xc[:, :, :tn]
                _layernorm(nc, lnw, psln1, src_ln1, N, ln1g_sb, ln1b_sb,
                           eps_sb, ones, hT,
                           chunk_list=[(0, 256), (256, 256), (512, 512),
                                       (1024, 346)])

                wv_sb = wvp.tile([P, CT, C], F8)
                nc.sync.dma_start(wv_sb, d["wv"][:, :, :])

                wms = {}

                def qk_mm(m, qoff, qn):
                    if m not in wms:
                        wms[m] = pre_wm.pop(m) if m in pre_wm else load_wqk(m)
                    dest = QTt if m < 8 else KTt
                    ps = psA.tile([P, 512], F32, tag="ps", name=f"ps{m}_{qoff}")
                    for k in range(CT // 2):
                        nc.tensor.matmul(ps[:, :qn],
                                         wms[m][:, 2 * k:2 * k + 2, :],
                                         hT[:, 2 * k:2 * k + 2, qoff:qoff + qn],
                                         start=(k == 0), stop=(k == CT // 2 - 1),
                                         perf_mode=mybir.MatmulPerfMode.DoubleRow)
                    s2 = (DH ** -0.5) / WS if m < 8 else 1.0 / WS
                    nc.vector.tensor_scalar(dest[:, m % 8, qoff:qoff + qn],
                                            ps[:, :qn],
                                            bqk_sb[:, m:m + 1], s2, ADD, MUL)

                def v_mm(t):
                    tp = min(P, N - t * P)
                    ps = psV.tile([P, 2, 512], F32, tag="psv", name=f"psv{t}")
                    for vc in range(2):
                        for k in range(CT // 2):
                            nc.tensor.matmul(ps[:tp, vc, :],
                                             hT[:, 2 * k:2 * k + 2, t * P:t * P + tp],
                                             wv_sb[:, 2 * k:2 * k + 2,
                                                   vc * 512:(vc + 1) * 512],
                                             start=(k == 0), stop=(k == CT // 2 - 1),
                                             perf_mode=mybir.MatmulPerfMode.DoubleRow)
                    # evac on ACT so AV matmuls depend on one engine only
                    # (b_v is folded into the proj bias on the host)
                    nc.scalar.mul(
                        vaug[:tp, t, :, :DH],
                        ps[:tp, :, :].rearrange("p v (h dh) -> p (v h) dh", dh=DH),
                        1.0 / WS)

                # wave 0: tokens [0,512) ready first
                for m in range(8):
                    qk_mm(m, 0, 512)
                for m in range(8, 16):
                    qk_mm(m, 0, 512)
                for t in range(4):
                    v_mm(t)
                # wave 1: tokens [512,1024)
                for m in range(8):
                    qk_mm(m, 512, Q - 512)
                for m in range(8, 16):
                    qk_mm(m, 512, 512)
                for t in range(4, 8):
                    v_mm(t)
                # wave 2: tokens [1024,1370)
                for m in range(8, 16):
                    qk_mm(m, 1024, N - 1024)
                for t in range(8, KT):
                    v_mm(t)

            # ---------- Phase C: attention ----------
            # (xq / wproj loads issued first so they overlap attention)
            xqp = tc.alloc_tile_pool(name="xqp", bufs=1)
            wfc1_sb = xqp.tile([P, CT, HID], F8)
            nc.sync.dma_start(wfc1_sb, d["wfc1"][:, :, :])
            prx = tc.alloc_tile_pool(name="prx", bufs=1)
            xq = prx.tile([P, CT, Q], F32)
            nc.sync.dma_start(xq, d["xt"][:, :, :Q])
            wproj_sb = prx.tile([P, CT, C], F8)
            nc.sync.dma_start(wproj_sb, d["wproj"][:, :, :])

            GROUPS = [[0, 1, 2], [3, 4, 5], [6, 7, 8], [9, 10]]
            with tc.tile_pool(name="pss", bufs=2, space="PSUM") as pss, \
                 tc.tile_pool(name="psav", bufs=2, space="PSUM") as psav, \
                 tc.tile_pool(name="ptp", bufs=3) as ptp, \
                 tc.tile_pool(name="nrmd", bufs=2, space="DRAM") as nrmd, \
                 tc.tile_pool(name="nrm", bufs=2) as nrm:
                work_items = [(h, qoff, qn) for h in range(H)
                              for (qoff, qn) in QCH]
                pending = []  # (pt, grp, h, qn, ps_av)

                def flush_av(pending):
                    pt, grp, h, qn, ps_av = pending.pop(0)
                    for jj, j in enumerate(grp):
                        tp = min(P, N - j * P)
                        nc.tensor.matmul(ps_av[:DH + 1, :qn],
                                         vaug[:tp, j, h, :],
                                         pt[:tp, jj, :qn],
                                         start=(j == 0), stop=(j == KT - 1))

                av_tiles = {}
                for wi, (h, qoff, qn) in enumerate(work_items):
                    ht_, hoff = h // 2, (h % 2) * DH
                    ps_av = psav.tile([P, 512], F32, tag="av", name=f"av{wi}")
                    av_tiles[wi] = (ps_av, h, ht_, hoff, qoff, qn)
                    for gi, grp in enumerate(GROUPS):
                        ps_s = pss.tile([P, 3, 512], F32, tag="s",
                                        name=f"s{wi}_{gi}")
                        if gi == len(GROUPS) - 1:
                            # pad the ragged tile's rows so one exp call
                            # covers the group (exp(-30)~=0); full partition
                            # range (PSUM wants 32-aligned offsets), the
                            # matmul below then overwrites rows [0, 90)
                            nc.vector.memset(ps_s[:, 1, :qn], -30.0)
                        for jj, j in enumerate(grp):
                            tp = min(P, N - j * P)
                            nc.tensor.matmul(
                                ps_s[:tp, jj, :qn],
                                KTt[hoff:hoff + DH, ht_, j * P:j * P + tp],
                                QTt[hoff:hoff + DH, ht_, qoff:qoff + qn],
                                start=True, stop=True)
                        pt = ptp.tile([P, 3, 512], BF16, tag="pt",
                                      name=f"pt{wi}_{gi}")
                        nc.scalar.activation(pt[:, :len(grp), :qn],
                                             ps_s[:, :len(grp), :qn], AF.Exp)
                        pending.append((pt, grp, h, qn, ps_av))
                        if len(pending) > 2:
                            flush_av(pending)
                    # normalize the item whose AV chain completed
                    done = wi - 1 if wi > 0 else None
                    if wi == len(work_items) - 1:
                        while pending:
                            flush_av(pending)
                        done_list = [wi - 1, wi] if wi > 0 else [wi]
                    elif done is not None:
                        done_list = [done]
                    else:
                        done_list = []
                    for dwi in done_list:
                        pav, dh_, dht, dhoff, dqoff, dqn = av_tiles.pop(dwi)
                        rrow = nrm.tile([1, 512], F32, tag="rrow",
                                        name=f"rr{dwi}")
                        nc.vector.reciprocal(rrow[:, :dqn],
                                             pav[DH:DH + 1, :dqn])
                        rdram = nrmd.tile([1, 512], F32, tag="rd",
                                          name=f"rd{dwi}")
                        nc.sync.dma_start(rdram[:, :dqn], rrow[:, :dqn])
                        rbc = nrm.tile([DH, 512], F32, tag="rbc",
                                       name=f"rb{dwi}")
                        nc.sync.dma_start(rbc[:, :dqn],
                                          _pbroadcast(rdram[:1, :dqn], DH))
                        nc.vector.tensor_tensor(
                            oT[dhoff:dhoff + DH, dht, dqoff:dqoff + dqn],
                            pav[:DH, :dqn], rbc[:, :dqn], MUL)

            # ---------- Phase D: proj + residual + LN2 ----------
            with tc.tile_pool(name="prw", bufs=2) as prw, \
                 tc.tile_pool(name="psln2", bufs=2, space="PSUM") as psln2, \
                 tc.tile_pool(name="pspr", bufs=4, space="PSUM") as pspr:
                # qc outer so x1T's first chunk completes early (LN2 can start)
                for (qoff, qn) in QCH:
                    for m in range(CT):
                        ps = pspr.tile([P, 512], F32, tag="ps")
                        for k in range(CT // 2):
                            nc.tensor.matmul(ps[:, :qn],
                                             wproj_sb[:, 2 * k:2 * k + 2,
                                                      m * P:(m + 1) * P],
                                             oT[:, 2 * k:2 * k + 2, qoff:qoff + qn],
                                             start=(k == 0), stop=(k == CT // 2 - 1),
                                             perf_mode=mybir.MatmulPerfMode.DoubleRow)
                        tmp = prw.tile([P, 512], F32, tag="prtmp")
                        nc.vector.tensor_scalar(tmp[:, :qn], ps[:, :qn],
                                                g1s_sb[:, m:m + 1],
                                                bproj_sb[:, m:m + 1], MUL, ADD)
                        nc.gpsimd.tensor_add(x1T[:, m, qoff:qoff + qn], tmp[:, :qn],
                                             xq[:, m, qoff:qoff + qn])
                _layernorm(nc, prw, psln2,
                           lambda toff, tn: x1T[:, :, toff:toff + tn],
                           Q, ln2g_sb, ln2b_sb, eps_sb, ones, h2T,
                           chunk_list=QCM)
            prx.release()

            # ---------- Phase E: MLP + residual ----------
            with tc.tile_pool(name="f2w", bufs=2) as f2w, \
                 tc.tile_pool(name="gel", bufs=1) as gel, \
                 tc.tile_pool(name="outp", bufs=2) as outp, \
                 tc.tile_pool(name="psml", bufs=2, space="PSUM") as psml, \
                 tc.tile_pool(name="psm2", bufs=4, space="PSUM") as psm2:
                geluT = gel.tile([P, HT, 2, 352], F8)
                for m in range(HT):
                    ps = psml.tile([P, 2, 512], F32, tag="ps2", name=f"ps2_{m}")
                    for k in range(CT // 2):
                        for ci, (qoff, qn) in enumerate(QCM):
                            nc.tensor.matmul(ps[:, ci, :qn],
                                             wfc1_sb[:, 2 * k:2 * k + 2,
                                                     m * P:(m + 1) * P],
                                             h2T[:, 2 * k:2 * k + 2, qoff:qoff + qn],
                                             start=(k == 0), stop=(k == CT // 2 - 1),
                                             perf_mode=mybir.MatmulPerfMode.DoubleRow)
                    nc.vector.memset(ps[:, 1, QCM[1][1]:], 0.0)
                    nc.scalar.activation(geluT[:, m, :, :343],
                                         ps[:, :, :343], AF.Gelu,
                                         bias=bfc1_sb[:, m:m + 1], scale=1.0 / WS)
                for m in range(CT):
                    w2 = f2w.tile([P, HT, P], F8, tag="w2")
                    nc.sync.dma_start(w2, d["wfc2"][m])
                    om = outp.tile([P, Q], F32, tag="om")
                    pss_ = [psm2.tile([P, 512], F32, tag="ps", name=f"psml{ci}") for ci in range(len(QCH))]
                    for k in range(HT // 2):
                        for ci, (qoff, qn) in enumerate(QCM):
                            nc.tensor.matmul(pss_[ci][:, :qn],
                                             w2[:, 2 * k:2 * k + 2, :],
                                             geluT[:, 2 * k:2 * k + 2, ci, :qn],
                                             start=(k == 0), stop=(k == HT // 2 - 1),
                                             perf_mode=mybir.MatmulPerfMode.DoubleRow)
                    for ci, (qoff, qn) in enumerate(QCM):
                        tmp = outp.tile([P, 512], F32, tag="f2tmp",
                                        name=f"f2tmp{ci}")
                        nc.vector.tensor_scalar(tmp[:, :qn], pss_[ci][:, :qn],
                                                g2s_sb[:, m:m + 1],
                                                bfc2_sb[:, m:m + 1], MUL, ADD)
                        nc.gpsimd.tensor_add(om[:, qoff:qoff + qn], tmp[:, :qn],
                                             x1T[:, m, qoff:qoff + qn])
                    nc.sync.dma_start(out_d[:, m, :], om[:, :])
            xqp.release()

    _legalize_matmul_waits(nc)
    return nc


_PROGRAM = None


def _get_program():
    global _PROGRAM
    if _PROGRAM is None:
        _PROGRAM = _build_program()
    return _PROGRAM


def _ptile(w, n_out_tiles, n_in_tiles, dtype=None):
    """[Cin, Cout] -> [m, p, k, col] pretiled lhsT layout."""
    a = w.reshape(n_in_tiles, P, n_out_tiles, P)
    return np.ascontiguousarray(a.transpose(2, 1, 0, 3)).astype(
        dtype if dtype is not None else ml_dtypes.bfloat16)


def _col_layout(v):
    """[D] -> [P, D//P] with column j = dims j*128..j*128+127."""
    return np.ascontiguousarray(v.reshape(-1, P).T).astype(np.float32)


def prepare_inputs(x, ln1_g, ln1_b, w_qkv, b_qkv, w_proj, b_proj, gamma1,
                   ln2_g, ln2_b, w_fc1, b_fc1, w_fc2, b_fc2, gamma2):
    """Host-side prep: returns (shared weight map, per-core input maps)."""
    wqkvT = np.ascontiguousarray(w_qkv.T).astype(np.float32)  # [C, 3C]
    b_qkv = np.asarray(b_qkv, np.float32)
    gamma1 = np.asarray(gamma1, np.float32)
    gamma2 = np.asarray(gamma2, np.float32)

    # fp8 weights are stored scaled by WS (unscaled at PSUM evacuation);
    # the attention 1/sqrt(dh) and the layer-scale gammas are applied at
    # evacuation time too (folding them here would denormalize e4m3)
    wm = {}
    wm["wqk"] = _ptile(wqkvT[:, :2 * C] * WS, 16, CT, F8NP)
    wm["bqk"] = _col_layout(b_qkv[:2 * C] * WS)
    wv = np.ascontiguousarray(wqkvT[:, 2 * C:])  # [C, C]
    wm["wv"] = np.ascontiguousarray(
        (wv * WS).reshape(CT, P, C).transpose(1, 0, 2)).astype(F8NP)
    wprojT = np.asarray(w_proj, np.float32).T
    wm["wproj"] = np.ascontiguousarray(
        (wprojT * WS).reshape(CT, P, C).transpose(1, 0, 2)).astype(F8NP)
    # b_v passes through softmax unchanged (convex combination), fold it here
    b_v = b_qkv[2 * C:]
    bproj_eff = (np.asarray(b_proj, np.float32)
                 + b_v @ np.asarray(w_proj, np.float32).T)
    wm["bproj"] = _col_layout(bproj_eff * gamma1)
    wm["g1s"] = _col_layout(gamma1 / WS)
    wm["g2s"] = _col_layout(gamma2 / WS)
    wm["ln1g"] = _col_layout(np.asarray(ln1_g, np.float32))
    wm["ln1b"] = _col_layout(np.asarray(ln1_b, np.float32))
    wm["ln2g"] = _col_layout(np.asarray(ln2_g, np.float32))
    wm["ln2b"] = _col_layout(np.asarray(ln2_b, np.float32))
    wfc1T = np.asarray(w_fc1, np.float32).T * WS
    wm["wfc1"] = np.ascontiguousarray(
        wfc1T.reshape(CT, P, HID).transpose(1, 0, 2)).astype(F8NP)
    wm["bfc1"] = _col_layout(np.asarray(b_fc1, np.float32))
    wm["wfc2"] = _ptile(np.asarray(w_fc2, np.float32).T * WS, CT, HT, F8NP)
    wm["bfc2"] = _col_layout(np.asarray(b_fc2, np.float32) * gamma2)

    in_maps = []
    x = np.asarray(x, np.float32)
    for core in range(NCORES):
        b, t = core // 2, core % 2
        xb = np.roll(x[b], -t * Q, axis=0)  # queries become tokens [0, Q)
        xtl = np.ascontiguousarray(
            xb.T.reshape(CT, P, N).transpose(1, 0, 2)).astype(np.float32)
        m = dict(wm)
        m["xt"] = xtl
        in_maps.append(m)
    return in_maps


def gather_output(results):
    out = np.empty((B, N, C), np.float32)
    for core in range(NCORES):
        b, t = core // 2, core % 2
        o = results[core]["out"]  # [P, CT, Q]
        out[b, t * Q:(t + 1) * Q, :] = o.transpose(1, 0, 2).reshape(C, Q).T
    return out


def kernel(**inputs):
    nc = _get_program()
    in_maps = prepare_inputs(**{k: np.asarray(v) for k, v in inputs.items()})
    res = run_bass_kernel_spmd(nc, in_maps, list(range(NCORES)))
    return gather_output(res.results)


if __name__ == "__main__":
    _get_program()
    print("program built OK")
